# revision 1
# baseline (speedup 1.0000x reference)
"""Trainium2 Bass kernel for nn_MeshCrossAttention (mesh cross-attention + per-head MLP).

Sharding: data-parallel over batch B=16 -> 2 batches per NeuronCore, 8 cores,
no collectives.

Design (vs the ~1.33 ms v1 baseline; measures ~0.51 ms):
  - bf16 operands everywhere on the PE (fp32 PSUM accumulate); all weights
    RESIDENT in SBUF (loaded once), activations streamed -> PE never waits
    on weight DMA, which keeps the HAM clock at full rate.
  - Every matmul is a FULL 128x128 tile (partial-K/M matmuls measure ~2x
    slower on HW): q is stored zero-padded per head (the paired head's 64
    rows are zero) so K=64 scores run as 128x128x512 against the compact
    two-head kT tiles; MLP1/MLP2 use block-diagonal [128,128] weights to
    process HEAD PAIRS in one stream; Gelu also runs per pair.
  - Scores stay transposed (sT [LK, LQ]); exp on ScalarE -> eT bf16 tiles.
    Context accumulates in NATURAL layout: lhsT = eT chunk, rhs = va slice
    [LK, HD+1] whose ones column makes the softmax denominator a PER
    PARTITION scalar -> normalize is one DVE reciprocal + one fused
    broadcast multiply (no DRAM-roundtrip partition broadcast).
  - Per head: 12 (score->exp) steps; ctx matmuls consume exp output three
    steps delayed and the previous pair's cat transposes (PE identity
    matmuls) are interleaved as filler, so the PE runs through ScalarE's
    exp latency. Exp/Gelu batch in 8-head phases (4 table swaps per batch)
    with dummy activations prefetching the table.
  - PSUM: big(2: proj/transposes/mlp1) + att(3: scores/mlp2) + ctx(3,
    paired [128, 390]) = 8 banks, one start/stop per bank per group.
"""
import math
import sys

import numpy as np

if "/opt/trn_rl_repo" not in sys.path:
    sys.path.insert(0, "/opt/trn_rl_repo")

import ml_dtypes  # noqa: E402

import concourse.bass as bass  # noqa: E402
import concourse.tile as tile  # noqa: E402
from concourse import bacc, mybir  # noqa: E402
from concourse.bass_utils import run_bass_kernel_spmd  # noqa: E402

F32 = mybir.dt.float32
BF16 = mybir.dt.bfloat16

D, H, HD, J = 1024, 16, 64, 3
B, LQ, LK = 16, 512, 512
P = 128
N_CORES = 8
B_LOC = B // N_CORES  # 2
E = HD + 1            # 65: head stripe width in va (ones column at HD)
HG = 8                # heads per exp/gelu phase group


def _emit(tc, aps, dbg=False):
    nc = tc.nc
    ctx_mgr = []

    def pool(name, bufs, space="SBUF"):
        p = tc.tile_pool(name=name, bufs=bufs, space=space)
        ctx_mgr.append(p)
        return p.__enter__()

    const = pool("const", 1)
    ain = pool("ain", 16)          # streamed activation chunks [128, 512] bf16
    qt_pool = pool("qt", 24)       # zero-padded per-head qT tiles [128, 512]
    kt_pool = pool("kt", 24)
    va_pool = pool("va", 3)
    expp = pool("expp", 12)
    recp = pool("recp", 4)
    catp = pool("catp", 10)        # pair cat tiles [128, 384]
    ctp = pool("ctp", 15)          # catTj pair tiles, alive until mini-MLP
    h1p = pool("h1p", 4)
    ostg = pool("ostg", 5)

    ps_big = pool("ps_big", 2, "PSUM")   # proj accum + cat transposes + mlp1
    ps_att = pool("ps_att", 3, "PSUM")   # scores / mlp2 [128, 512]
    ps_ctx = pool("ps_ctx", 3, "PSUM")   # ctx pairs [128, 2*J*E = 390]

    # ---------------- resident constants ----------------
    # Only wq/bq are DMAed up front (the first projection needs them); the
    # rest is deferred until after the Q-projection is emitted so the PE can
    # start ~8us earlier at kernel start.
    wq_sb, wk_sb, wv_sb = [], [], []
    for nm, lst in (("wqt", wq_sb), ("wkt", wk_sb), ("wvt", wv_sb)):
        for i in range(8):
            t = const.tile([P, D], BF16, tag=f"{nm}{i}", name=f"{nm}{i}")
            lst.append(t)
    for i in range(8):
        nc.sync.dma_start(out=wq_sb[i][:], in_=aps["wqt"][i * P:(i + 1) * P, :])
    bq_sb = const.tile([P, 8], F32, tag="bq", name="bq_sb")
    nc.sync.dma_start(out=bq_sb[:], in_=aps["bq"][:, :])

    w1jd = [const.tile([P, P], BF16, tag=f"w1jd{j}", name=f"w1jd{j}")
            for j in range(J)]
    w2bd = const.tile([P, P], BF16, tag="w2bd", name="w2bd")
    ident = const.tile([P, P], BF16, tag="ident", name="ident")
    bk_sb = const.tile([P, 8], F32, tag="bk", name="bk_sb")
    bv_bc = const.tile([P, D], BF16, tag="bv", name="bv_bc")
    b2_bc = const.tile([P, D], F32, tag="b2", name="b2_bc")
    b1_sb = const.tile([P, 1], F32, tag="b1", name="b1_sb")
    tbl_scr = const.tile([P, 1], F32, tag="tbl", name="tbl_scr")

    def deferred_const_dmas():
        for i in range(8):
            nc.sync.dma_start(out=wk_sb[i][:],
                              in_=aps["wkt"][i * P:(i + 1) * P, :])
        nc.sync.dma_start(out=bk_sb[:], in_=aps["bk"][:, :])
        for i in range(8):
            nc.sync.dma_start(out=wv_sb[i][:],
                              in_=aps["wvt"][i * P:(i + 1) * P, :])
        nc.sync.dma_start(out=bv_bc[:], in_=aps["bv_bc"][:, :])
        for j in range(J):
            nc.sync.dma_start(out=w1jd[j][:], in_=aps["w1jd"][j])
        nc.sync.dma_start(out=w2bd[:], in_=aps["w2bd"][:, :])
        nc.sync.dma_start(out=ident[:], in_=aps["ident"][:, :])
        nc.sync.dma_start(out=b2_bc[:], in_=aps["b2_bc"][:, :])
        nc.sync.dma_start(out=b1_sb[:], in_=aps["b1"][:, :])

    def load_acts(ap_slice):
        ts = []
        for ic in range(8):
            t = ain.tile([P, 512], BF16, tag="ain", name="act")
            nc.sync.dma_start(out=t[:], in_=ap_slice[ic * P:(ic + 1) * P, :])
            ts.append(t)
        return ts

    def proj_T(w_tiles, x_tiles, bias_sb, out_pool, out_tag):
        """out[oc] [128, 512] = (W @ x^T) chunk + bias, bf16."""
        outs = []
        for oc in range(8):
            pss = ps_big.tile([P, 512], F32, tag="big", name="pss")
            for ic in range(8):
                nc.tensor.matmul(
                    out=pss[:], lhsT=w_tiles[ic][:, oc * P:(oc + 1) * P],
                    rhs=x_tiles[ic][:], start=(ic == 0), stop=(ic == 7))
            t = out_pool.tile([P, 512], BF16, tag=out_tag, name=out_tag)
            nc.vector.tensor_scalar_add(t[:], pss[:], bias_sb[:, oc:oc + 1])
            outs.append(t)
        return outs

    def proj_Q(x_tiles):
        """Q projection into zero-padded per-head tiles qtp[h] [128, 512]:
        head h's 64 q-dims at rows (h%2)*64, the other 64 rows ZERO, so the
        scores matmul is a full 128x128x512 tile against the compact kT
        (the zero q rows null the paired head's k contribution)."""
        outs = []
        for oc in range(8):
            pss = ps_big.tile([P, 512], F32, tag="big", name="pss")
            for ic in range(8):
                nc.tensor.matmul(
                    out=pss[:], lhsT=wq_sb[ic][:, oc * P:(oc + 1) * P],
                    rhs=x_tiles[ic][:], start=(ic == 0), stop=(ic == 7))
            te = qt_pool.tile([P, 512], BF16, tag="qt", name="qtp_e")
            to = qt_pool.tile([P, 512], BF16, tag="qt", name="qtp_o")
            nc.gpsimd.memset(te[HD:P, :], 0.0)
            nc.gpsimd.memset(to[0:HD, :], 0.0)
            nc.vector.tensor_scalar_add(te[0:HD, :], pss[0:HD, :],
                                        bq_sb[0:HD, oc:oc + 1])
            nc.vector.tensor_scalar_add(to[HD:P, :], pss[HD:P, :],
                                        bq_sb[HD:P, oc:oc + 1])
            outs += [te, to]
        return outs

    def proj_V(x_tiles, va, defer_last=False):
        """va [128, 4, H*E] natural head-interleaved V + ones column.
        With defer_last, the half=1 groups (heads 8-15, first needed by the
        second head-group) are returned as thunks to seed the attention
        phase's filler queue instead of being emitted inline."""
        nc.sync.dma_start(
            out=va.rearrange("p c (h e) -> p c h e", e=E)[:, :, :, HD],
            in_=aps["ones_cols"][:, :, :])
        thunks = []
        for half in range(2):
            for nck in range(4):
                pss = ps_big.tile([P, 512], F32, tag="big", name="pssv")

                def mm(ic, pss=pss, half=half, nck=nck):
                    nc.tensor.matmul(
                        out=pss[:],
                        lhsT=x_tiles[ic][:, nck * P:(nck + 1) * P],
                        rhs=wv_sb[ic][:, half * 512:(half + 1) * 512],
                        start=(ic == 0), stop=(ic == 7))

                def add(pss=pss, half=half, nck=nck):
                    dst = va[:, nck, :].rearrange("p (h e) -> p h e", e=E)[
                        :, half * 8:(half + 1) * 8, 0:HD]
                    nc.vector.tensor_tensor(
                        out=dst,
                        in0=pss[:].rearrange("p (h e) -> p h e", e=HD),
                        in1=bv_bc[:, half * 512:(half + 1) * 512].rearrange(
                            "p (h e) -> p h e", e=HD),
                        op=mybir.AluOpType.add)

                if defer_last and half == 1:
                    for ic in range(8):
                        thunks.append(lambda ic=ic, f=mm: f(ic))
                    thunks.append(lambda f=add: f())
                else:
                    for ic in range(8):
                        mm(ic)
                    add()
        return thunks

    for b in range(B_LOC):
        # ================= projections =================
        qin = load_acts(aps["qt_in"][b])
        qtp = proj_Q(qin)
        if b == 0:
            deferred_const_dmas()

        kT = []
        for j in range(J):
            kin = load_acts(aps["kt_in"][j, b])
            kT.append(proj_T(wk_sb, kin, bk_sb, kt_pool, "kt"))

        va_list = []
        vthunks = []
        for j in range(J):
            vin = load_acts(aps["vt_in"][j, b])
            va = va_pool.tile([P, 4, H * E], BF16, tag="va", name="va")
            vthunks = proj_V(vin, va, defer_last=(j == J - 1))
            va_list.append(va)

        if dbg and b == 0:
            for oc in range(8):
                nc.sync.dma_start(out=aps["dbg_qt"][oc], in_=qtp[oc][:])
                nc.sync.dma_start(out=aps["dbg_kt0"][oc], in_=kT[0][oc][:])
            nc.sync.dma_start(out=aps["dbg_va0"][:, :, :], in_=va_list[0][:])

        ost = [ostg.tile([P, D], F32, tag="ostg", name=f"ost{i}")
               for i in range(4)]

        # ================= attention + MLP, 8-head phases =================
        # Per head: 12 (score -> exp) steps; ctx matmuls consume the PREVIOUS
        # step's exp output so the PE never waits on ScalarE. The previous
        # head's cat transposes are interleaved as PE filler during exp
        # latency. Every PE matmul in this phase is a full 128x128 tile
        # (partial-K/M matmuls measure ~2x slower on HW): scores use the
        # zero-padded kT, the j2 cat columns are packed per head PAIR and
        # MLP1/MLP2 use block-diagonal weights over head pairs.
        fillers = vthunks
        for hg in range(H // HG):
            catT_all = {}
            catp_tiles = None
            for h in range(hg * HG, (hg + 1) * HG):
                pscs = [ps_ctx.tile([P, 2 * J * E], F32, tag="ctx",
                                    name=f"psc{i}") for i in range(2)]

                def emit_ctx(j, ci, et):
                    for lqc in range(4):
                        # One start/stop per PSUM bank: start lazily zeroes
                        # the whole 2KB bank, so only the first matmul
                        # touching each pair-tile starts the group.
                        nc.tensor.matmul(
                            out=pscs[lqc // 2][
                                :, (lqc % 2) * J * E + j * E:
                                (lqc % 2) * J * E + (j + 1) * E],
                            lhsT=et[:, lqc * P:(lqc + 1) * P],
                            rhs=va_list[j][:, ci, h * E:(h + 1) * E],
                            start=(j == 0 and ci == 0 and lqc % 2 == 0),
                            stop=(j == J - 1 and ci == 3 and lqc % 2 == 1))

                pend = []
                for j in range(J):
                    for ci in range(4):
                        pss = ps_att.tile([P, LQ], F32, tag="att", name="ps_s")
                        nc.tensor.matmul(
                            out=pss[:],
                            lhsT=kT[j][h // 2][:, ci * P:(ci + 1) * P],
                            rhs=qtp[h][:], start=True, stop=True)
                        et = expp.tile([P, LQ], BF16, tag="expp", name="et")
                        nc.scalar.activation(
                            out=et[:], in_=pss[:],
                            func=mybir.ActivationFunctionType.Exp,
                            scale=1.0 / math.sqrt(HD))
                        if dbg and b == 0 and h == 0 and j == 0:
                            nc.sync.dma_start(out=aps["dbg_exp"][ci], in_=et[:])
                        if fillers:
                            fillers.pop(0)()
                        pend.append((j, ci, et))
                        if len(pend) > 3:
                            emit_ctx(*pend.pop(0))
                while pend:
                    emit_ctx(*pend.pop(0))

                # normalize: denominator is column HD of each head stripe.
                # Each j-chunk goes into the head PAIR's shared catp tile
                # (64 columns per head) so the transposes and MLP1 run as
                # full 128x128 tiles with block-diagonal weights.
                if h % 2 == 0:
                    catp_tiles = [catp.tile([P, J * P], BF16, tag="catp",
                                            name="catp") for _ in range(4)]
                for pi in range(2):
                    rec = recp.tile([P, 2 * J], F32, tag="rec", name="rec")
                    nc.vector.reciprocal(
                        rec[:],
                        pscs[pi].rearrange("p (x e) -> p x e", e=E)[:, :, HD])
                    for half in range(2):
                        lqc = pi * 2 + half
                        # one fused multiply: [128, 3, 64] x per-(row,j)
                        # reciprocal broadcast along the last dim
                        nc.vector.tensor_tensor(
                            out=catp_tiles[lqc].rearrange(
                                "p (x e) -> p x e", e=P)[
                                :, :, (h % 2) * HD:(h % 2) * HD + HD],
                            in0=pscs[pi].rearrange(
                                "p (x e) -> p x e", e=E)[
                                :, half * J:(half + 1) * J, 0:HD],
                            in1=rec[:, half * J:(half + 1) * J].unsqueeze(2)
                            .to_broadcast((P, J, HD)),
                            op=mybir.AluOpType.mult)
                    if dbg and b == 0 and h == 0:
                        nc.sync.dma_start(out=aps["dbg_rec"][pi], in_=rec[:])
                if dbg and b == 0 and h == 1:
                    for lqc in range(4):
                        nc.sync.dma_start(out=aps["dbg_cat"][lqc],
                                          in_=catp_tiles[lqc][:, 0:P])

                # After the odd head, queue the pair's transpose work as PE
                # filler for the next head's exp-latency slots.
                if h % 2 == 1:
                    thunks = []
                    catTs = []
                    for j in range(J):
                        ptj = ps_big.tile([P, LQ], F32, tag="big", name="ptj")
                        catTj = ctp.tile([P, LQ], BF16, tag="ct", name="catTj")
                        for lqc in range(4):
                            thunks.append(
                                lambda lqc=lqc, ptj=ptj, j=j,
                                c=catp_tiles[lqc]:
                                nc.tensor.matmul(
                                    out=ptj[:, lqc * P:(lqc + 1) * P],
                                    lhsT=c[:, j * P:(j + 1) * P], rhs=ident[:],
                                    start=(lqc == 0), stop=(lqc == 3)))
                        thunks.append(
                            lambda ptj=ptj, catTj=catTj:
                            nc.vector.tensor_copy(out=catTj[:], in_=ptj[:]))
                        catTs.append(catTj)
                    catT_all[h // 2] = catTs
                    fillers.extend(thunks)
            while fillers:
                fillers.pop(0)()

            # ---- MLP1 / Gelu / MLP2 per head pair, full 128-tiles ----
            # tiny dummy activation pulls the Gelu table in while the PE is
            # still busy with transposes, instead of stalling the first gelu
            nc.scalar.activation(out=tbl_scr[:], in_=b1_sb[:],
                                 func=mybir.ActivationFunctionType.Gelu)
            for pi in range(hg * HG // 2, (hg + 1) * HG // 2):
                ph1p = ps_big.tile([P, LQ], F32, tag="big", name="ph1p")
                for j in range(J):
                    nc.tensor.matmul(out=ph1p[:, :], lhsT=w1jd[j][:],
                                     rhs=catT_all[pi][j][:],
                                     start=(j == 0), stop=(j == J - 1))
                h1 = h1p.tile([P, LQ], BF16, tag="h1", name="h1")
                nc.scalar.activation(
                    out=h1[:], in_=ph1p[:],
                    func=mybir.ActivationFunctionType.Gelu, bias=b1_sb[:])
                if dbg and b == 0 and pi == 0:
                    nc.sync.dma_start(out=aps["dbg_catT0"][:, :],
                                      in_=catT_all[0][0][:])
                    nc.sync.dma_start(out=aps["dbg_h1"][:, :], in_=h1[:])
                def mlp2_thunk(lqc, h1=h1, pi=pi):
                    ps2 = ps_att.tile([P, LQ], F32, tag="att", name="ps2")
                    nc.tensor.matmul(
                        out=ps2[:, 0:P], lhsT=h1[:, lqc * P:(lqc + 1) * P],
                        rhs=w2bd[:], start=True, stop=True)
                    nc.vector.tensor_add(
                        ost[lqc][:, pi * P:(pi + 1) * P], ps2[:, 0:P],
                        b2_bc[:, pi * P:(pi + 1) * P])
                for lqc in range(4):
                    fillers.append(lambda lqc=lqc, f=mlp2_thunk: f(lqc))
            # preload the Exp table for the next attention phase
            nc.scalar.activation(out=tbl_scr[:], in_=b1_sb[:],
                                 func=mybir.ActivationFunctionType.Exp)

        while fillers:
            fillers.pop(0)()
        for lqc in range(4):
            nc.sync.dma_start(out=aps["out"][b, lqc * P:(lqc + 1) * P, :],
                              in_=ost[lqc][:])


    for p in reversed(ctx_mgr):
        p.__exit__(None, None, None)


_CACHE = {}


def _build(dbg=False):
    key = ("nc", dbg)
    if key in _CACHE:
        return _CACHE[key]
    nc = bacc.Bacc("TRN2", target_bir_lowering=False, debug=False)
    shapes = {
        "qt_in": ([B_LOC, D, LQ], BF16),
        "kt_in": ([J, B_LOC, D, LK], BF16),
        "vt_in": ([J, B_LOC, D, LK], BF16),
        "wqt": ([D, D], BF16),
        "wkt": ([D, D], BF16),
        "wvt": ([D, D], BF16),
        "w1jd": ([J, P, P], BF16),
        "w2bd": ([P, P], BF16),
        "ident": ([P, P], BF16),
        "ones_cols": ([P, 4, H], BF16),
        "bq": ([P, 8], F32),
        "bk": ([P, 8], F32),
        "bv_bc": ([P, D], BF16),
        "b2_bc": ([P, D], F32),
        "b1": ([P, 1], F32),
    }
    aps = {k: nc.dram_tensor(k, s, dt, kind="ExternalInput").ap()
           for k, (s, dt) in shapes.items()}
    aps["out"] = nc.dram_tensor("out", [B_LOC, LQ, D], F32,
                                kind="ExternalOutput").ap()
    if dbg:
        dbg_shapes = {
            "dbg_qt": ([8, P, 512], BF16), "dbg_kt0": ([8, P, 512], BF16),
            "dbg_va0": ([P, 4, H * E], BF16), "dbg_exp": ([4, P, LQ], BF16),
            "dbg_rec": ([2, P, 2 * J], F32), "dbg_cat": ([4, P, P], BF16),
            "dbg_catT0": ([P, LQ], BF16), "dbg_h1": ([P, LQ], BF16),
        }
        for k, (shp, dt) in dbg_shapes.items():
            aps[k] = nc.dram_tensor(k, shp, dt, kind="ExternalOutput").ap()
    with tile.TileContext(nc) as tc:
        _emit(tc, aps, dbg=dbg)
    nc.compile()
    _CACHE[key] = nc
    return nc


def _prep_in_maps(inputs):
    f32 = np.float32
    bf16 = ml_dtypes.bfloat16
    q = np.ascontiguousarray(np.asarray(inputs["query_states"], f32))
    k = np.ascontiguousarray(np.asarray(inputs["key_states"], f32))
    v = np.ascontiguousarray(np.asarray(inputs["value_states"], f32))
    Wq = np.asarray(inputs["Wq"], f32)
    Wk = np.asarray(inputs["Wk"], f32)
    Wv = np.asarray(inputs["Wv"], f32)
    W1 = np.asarray(inputs["W1"], f32)
    W2 = np.asarray(inputs["W2"], f32)
    bq = np.asarray(inputs["bq"], f32)
    bk = np.asarray(inputs["bk"], f32)
    bv = np.asarray(inputs["bv"], f32)
    b1 = np.asarray(inputs["b1"], f32)
    b2 = np.asarray(inputs["b2"], f32)

    wqt = np.ascontiguousarray(Wq.T).astype(bf16)
    wkt = np.ascontiguousarray(Wk.T).astype(bf16)
    wvt = np.ascontiguousarray(Wv.T).astype(bf16)
    W1T = np.ascontiguousarray(W1.T)                       # [192, 64]
    w1jd = np.zeros((J, P, P), f32)
    for j in range(J):
        blk = W1T[j * HD:(j + 1) * HD]                     # [64, 64]
        w1jd[j, :HD, :HD] = blk
        w1jd[j, HD:, HD:] = blk
    w1jd = w1jd.astype(bf16)
    W2T = np.ascontiguousarray(W2.T)                       # [64, 64]
    w2bd = np.zeros((P, P), f32)
    w2bd[:HD, :HD] = W2T
    w2bd[HD:, HD:] = W2T
    w2bd = w2bd.astype(bf16)
    ident = np.eye(P, dtype=f32).astype(bf16)
    bq_sb = np.ascontiguousarray(bq.reshape(8, P).T).astype(f32)
    bk_sb = np.ascontiguousarray(bk.reshape(8, P).T).astype(f32)
    bv_bc = np.tile(bv, (P, 1)).astype(bf16)
    b2_bc = np.tile(b2, (P, H)).astype(f32)
    b1_col = np.concatenate([b1, b1]).reshape(P, 1).astype(f32)
    ones_cols = np.ones((P, 4, H), f32).astype(bf16)

    qt_all = np.ascontiguousarray(q.transpose(0, 2, 1)).astype(bf16)
    kt_all = np.ascontiguousarray(k.transpose(0, 1, 3, 2)).astype(bf16)
    vt_all = np.ascontiguousarray(v.transpose(0, 1, 3, 2)).astype(bf16)

    in_maps = []
    for c in range(N_CORES):
        sl = slice(c * B_LOC, (c + 1) * B_LOC)
        in_maps.append({
            "qt_in": np.ascontiguousarray(qt_all[sl]),
            "kt_in": np.ascontiguousarray(kt_all[:, sl]),
            "vt_in": np.ascontiguousarray(vt_all[:, sl]),
            "wqt": wqt, "wkt": wkt, "wvt": wvt,
            "w1jd": w1jd, "w2bd": w2bd, "ident": ident,
            "ones_cols": ones_cols,
            "bq": bq_sb, "bk": bk_sb, "bv_bc": bv_bc,
            "b2_bc": b2_bc, "b1": b1_col,
        })
    return in_maps


def kernel(**inputs):
    nc = _build()
    in_maps = _prep_in_maps(inputs)
    res = run_bass_kernel_spmd(nc, in_maps, core_ids=list(range(N_CORES)))
    out = np.concatenate([res.results[i]["out"] for i in range(N_CORES)], axis=0)
    return out.astype(np.float32)



# revision 7
# speedup vs baseline: 1.0368x; 1.0368x over previous
"""Trainium2 Bass kernel for nn_MeshCrossAttention (mesh cross-attention + per-head MLP).

Sharding: data-parallel over batch B=16 -> 2 batches per NeuronCore, 8 cores,
no collectives.

v3 design (vs the 488us v2): the v2 kernel ran as serial per-batch phases:
projections (PE-bound, ScalarE idle ~100us/b) then attention (ScalarE
exp-bound at 100%, PE at ~80%).  Trace analysis: PE stream floor is ~339us
(matmul out-cols at 2.4GHz), ScalarE exp floor ~245us (578ns per [128,512]
exp tile, steady).  So the whole kernel is restructured as ONE software
pipeline where the PE never idles and exp overlaps everything:

  - attention is J-OUTER: sweep j=0..2 over all 16 heads per batch.  kT/va
    live per-j only, which frees enough SBUF to overlap the NEXT batch's
    projections with the current batch's attention.
  - all projection matmul groups after (q,k0,v0) of b0 are emitted as
    FILLER thunks from one global FIFO, popped between score matmuls at a
    tuned rate, so the PE streams projections while ScalarE exps scores.
  - ctx per (head,j) accumulates into ONE psum bank [128,4lqc,65] (ones
    column = softmax denominator per partition, as v2).  normalize is one
    reciprocal (DVE) + one fused broadcast-mult on GPSIMD (Pool engine,
    otherwise idle) into per-pair cat tiles [128,4,384].
  - during the last sweep (j=2) of each batch, the pair transposes + MLP1
    are enqueued as fillers right after each pair's normalize, so the
    filler queue never runs dry; gelu runs as one batch per b (2 act-table
    swaps per b instead of ~8).
  - mlp2 outputs stream to DRAM per [128,128] chunk (no big ost staging,
    no out-DMA tail).
"""
import math
import sys

import numpy as np

if "/opt/trn_rl_repo" not in sys.path:
    sys.path.insert(0, "/opt/trn_rl_repo")

import ml_dtypes  # noqa: E402

import concourse.bass as bass  # noqa: E402
import concourse.tile as tile  # noqa: E402
from concourse import bacc, mybir  # noqa: E402
from concourse.bass_utils import run_bass_kernel_spmd  # noqa: E402

F32 = mybir.dt.float32
BF16 = mybir.dt.bfloat16

D, H, HD, J = 1024, 16, 64, 3
B, LQ, LK = 16, 512, 512
P = 128
N_CORES = 8
B_LOC = B // N_CORES  # 2
E = HD + 1            # 65: head stripe width in va (ones column at HD)

# filler pops per emission point (tuned against the profile)
POP_HEAD = 8          # after each head's 4 score matmuls
POP_BOUND = 24        # at sweep boundaries
POP_MLP = 3           # between MLP-block emissions


def _emit(tc, aps, dbg=False):
    nc = tc.nc
    ctx_mgr = []

    def pool(name, bufs, space="SBUF"):
        p = tc.tile_pool(name=name, bufs=bufs, space=space)
        ctx_mgr.append(p)
        return p.__enter__()

    const = pool("const", 1)
    ain = pool("ain", 12)          # streamed activation chunks [128, 512] bf16
    qt_pool = pool("qt", 32)       # zero-padded per-head qT tiles [128, 512]
    kt_pool = pool("kt", 24)
    va_pool = pool("va", 3)
    expp = pool("expp", 10)
    recp = pool("recp", 4)
    catp = pool("catp", 8)         # pair cat tiles [128, 4, 384]
    ctp = pool("ctp", 8)           # catTj pair tiles [128, 512]
    h1p = pool("h1p", 4)
    ostg = pool("ostg", 6)         # mlp2 out staging [128, 128] f32

    ps_big = pool("ps_big", 2, "PSUM")   # proj accum + cat transposes + mlp1
    ps_att = pool("ps_att", 3, "PSUM")   # scores [128, 512] / mlp2 [128, 128]
    ps_ctx = pool("ps_ctx", 3, "PSUM")   # ctx per (h,j): [128, 4, 65]

    # ---------------- resident constants ----------------
    # wq/bq DMAed up front (first projection); the rest deferred so the PE
    # starts as early as possible.
    wq_sb, wk_sb, wv_sb = [], [], []
    for nm, lst in (("wqt", wq_sb), ("wkt", wk_sb), ("wvt", wv_sb)):
        for i in range(8):
            t = const.tile([P, D], BF16, tag=f"{nm}{i}", name=f"{nm}{i}")
            lst.append(t)
    bq_sb = const.tile([P, 8], F32, tag="bq", name="bq_sb")
    w1jd = [const.tile([P, P], BF16, tag=f"w1jd{j}", name=f"w1jd{j}")
            for j in range(J)]
    w2bd = const.tile([P, P], BF16, tag="w2bd", name="w2bd")
    ident = const.tile([P, P], BF16, tag="ident", name="ident")
    bk_sb = const.tile([P, 8], F32, tag="bk", name="bk_sb")
    bv_bc = const.tile([P, D], BF16, tag="bv", name="bv_bc")
    b2_bc = const.tile([P, D], F32, tag="b2", name="b2_bc")
    b1_sb = const.tile([P, 1], F32, tag="b1", name="b1_sb")
    tbl_scr = const.tile([P, 1], F32, tag="tbl", name="tbl_scr")

    def dma_qk_consts():
        for i in range(8):
            nc.sync.dma_start(out=wq_sb[i][:], in_=aps["wqt"][i * P:(i + 1) * P, :])
        nc.sync.dma_start(out=bq_sb[:], in_=aps["bq"][:, :])
        for i in range(8):
            nc.sync.dma_start(out=wk_sb[i][:], in_=aps["wkt"][i * P:(i + 1) * P, :])
        nc.sync.dma_start(out=bk_sb[:], in_=aps["bk"][:, :])

    def dma_rest_consts():
        for i in range(8):
            nc.sync.dma_start(out=wv_sb[i][:], in_=aps["wvt"][i * P:(i + 1) * P, :])
        nc.sync.dma_start(out=bv_bc[:], in_=aps["bv_bc"][:, :])
        for j in range(J):
            nc.sync.dma_start(out=w1jd[j][:], in_=aps["w1jd"][j])
        nc.sync.dma_start(out=w2bd[:], in_=aps["w2bd"][:, :])
        nc.sync.dma_start(out=ident[:], in_=aps["ident"][:, :])
        nc.sync.dma_start(out=b2_bc[:], in_=aps["b2_bc"][:, :])
        nc.sync.dma_start(out=b1_sb[:], in_=aps["b1"][:, :])

    def load_acts(ap_slice):
        ts = []
        for ic in range(8):
            t = ain.tile([P, 512], BF16, tag="ain", name="act")
            nc.sync.dma_start(out=t[:], in_=ap_slice[ic * P:(ic + 1) * P, :])
            ts.append(t)
        return ts

    # ---------------- projection emitters ----------------
    def proj_K_thunks(x_tiles, out_list):
        """out_list gets 8 tiles [128, 512] = (Wk @ x^T) + bias; returns
        thunk list (65 units: 64 matmuls + 8 adds merged into last units)."""
        thunks = []
        for oc in range(8):
            pss = ps_big.tile([P, 512], F32, tag="big", name="pssk")
            t = kt_pool.tile([P, 512], BF16, tag="kt", name="kt")
            out_list.append(t)

            def mm(ic, oc=oc, pss=pss):
                nc.tensor.matmul(
                    out=pss[:], lhsT=wk_sb[ic][:, oc * P:(oc + 1) * P],
                    rhs=x_tiles[ic][:], start=(ic == 0), stop=(ic == 7))

            def add(oc=oc, pss=pss, t=t):
                nc.vector.tensor_scalar_add(t[:], pss[:], bk_sb[:, oc:oc + 1])

            for ic in range(8):
                thunks.append(lambda ic=ic, f=mm: f(ic))
            thunks.append(lambda f=add: f())
        return thunks

    def proj_Q_thunks(x_tiles, out_list):
        """Zero-padded per-head qT tiles: head h rows at (h%2)*64, other 64
        rows zero (memset on gpsimd)."""
        thunks = []
        for oc in range(8):
            pss = ps_big.tile([P, 512], F32, tag="big", name="pssq")
            te = qt_pool.tile([P, 512], BF16, tag="qt", name="qtp_e")
            to = qt_pool.tile([P, 512], BF16, tag="qt", name="qtp_o")
            out_list.append(te)
            out_list.append(to)

            def mm(ic, oc=oc, pss=pss):
                nc.tensor.matmul(
                    out=pss[:], lhsT=wq_sb[ic][:, oc * P:(oc + 1) * P],
                    rhs=x_tiles[ic][:], start=(ic == 0), stop=(ic == 7))

            def add(oc=oc, pss=pss, te=te, to=to):
                nc.gpsimd.memset(te[HD:P, :], 0.0)
                nc.gpsimd.memset(to[0:HD, :], 0.0)
                nc.vector.tensor_scalar_add(te[0:HD, :], pss[0:HD, :],
                                            bq_sb[0:HD, oc:oc + 1])
                nc.vector.tensor_scalar_add(to[HD:P, :], pss[HD:P, :],
                                            bq_sb[HD:P, oc:oc + 1])

            for ic in range(8):
                thunks.append(lambda ic=ic, f=mm: f(ic))
            thunks.append(lambda f=add: f())
        return thunks

    def proj_V_thunks(x_tiles, va):
        """va [128, 4, H*E]: natural head-interleaved V + ones column."""
        nc.sync.dma_start(
            out=va.rearrange("p c (h e) -> p c h e", e=E)[:, :, :, HD],
            in_=aps["ones_cols"][:, :, :])
        thunks = []
        for half in range(2):
            for nck in range(4):
                pss = ps_big.tile([P, 512], F32, tag="big", name="pssv")

                def mm(ic, pss=pss, half=half, nck=nck):
                    nc.tensor.matmul(
                        out=pss[:],
                        lhsT=x_tiles[ic][:, nck * P:(nck + 1) * P],
                        rhs=wv_sb[ic][:, half * 512:(half + 1) * 512],
                        start=(ic == 0), stop=(ic == 7))

                def add(pss=pss, half=half, nck=nck):
                    dst = va[:, nck, :].rearrange("p (h e) -> p h e", e=E)[
                        :, half * 8:(half + 1) * 8, 0:HD]
                    nc.vector.tensor_tensor(
                        out=dst,
                        in0=pss[:].rearrange("p (h e) -> p h e", e=HD),
                        in1=bv_bc[:, half * 512:(half + 1) * 512].rearrange(
                            "p (h e) -> p h e", e=HD),
                        op=mybir.AluOpType.add)

                for ic in range(8):
                    thunks.append(lambda ic=ic, f=mm: f(ic))
                thunks.append(lambda f=add: f())
        return thunks

    # ---------------- global filler queue ----------------
    fillers = []

    def fill(n):
        for _ in range(n):
            if not fillers:
                return
            fillers.pop(0)()

    def flush_fillers():
        while fillers:
            fillers.pop(0)()

    # ---------------- attention sweep ----------------
    # per-batch persistent state
    qtp = {}    # b -> list of 16 padded q tiles
    kT = {}     # (b, j) -> list of 8 tiles
    va = {}     # (b, j) -> va tile
    catps = {}  # b -> list of 8 pair cat tiles [128, 4, 384]
    tmlp = {}   # b -> list of per-pair (catTs, ph1p) for gelu/mlp2 block

    def emit_ctx_norm(b, j, h, ets, dbg_tap):
        psc = ps_ctx.tile([P, 4, E], F32, tag="ctx", name="psc")
        for ci in range(4):
            for lqc in range(4):
                nc.tensor.matmul(
                    out=psc[:, lqc, :],
                    lhsT=ets[ci][:, lqc * P:(lqc + 1) * P],
                    rhs=va[(b, j)][:, ci, h * E:(h + 1) * E],
                    start=(ci == 0 and lqc == 0),
                    stop=(ci == 3 and lqc == 3))
        rec = recp.tile([P, 4], F32, tag="rec", name="rec")
        nc.vector.reciprocal(rec[:], psc[:, :, HD])
        nc.vector.tensor_tensor(
            out=catps[b][h // 2][:, :, j * P + (h % 2) * HD:
                                 j * P + (h % 2) * HD + HD],
            in0=psc[:, :, 0:HD],
            in1=rec[:].unsqueeze(2).to_broadcast((P, 4, HD)),
            op=mybir.AluOpType.mult)
        if dbg_tap:
            nc.sync.dma_start(out=aps["dbg_rec"][:, :], in_=rec[:])
            nc.sync.dma_start(out=aps["dbg_psc"][:, :, :], in_=psc[:])

    def sweep(b, j, enq_mlp=False):
        """j-outer attention sweep: 16 heads of scores->exp->ctx->normalize
        for mesh set j, popping fillers to keep the PE streaming."""
        pend = None  # (h, ets) awaiting ctx
        for h in range(H):
            ets = []
            for ci in range(4):
                pss = ps_att.tile([P, LQ], F32, tag="att", name="ps_s")
                nc.tensor.matmul(
                    out=pss[:],
                    lhsT=kT[(b, j)][h // 2][:, ci * P:(ci + 1) * P],
                    rhs=qtp[b][h], start=True, stop=True)
                et = expp.tile([P, LQ], BF16, tag="expp", name="et")
                nc.scalar.activation(
                    out=et[:], in_=pss[:],
                    func=mybir.ActivationFunctionType.Exp,
                    scale=1.0 / math.sqrt(HD))
                if dbg and b == 0 and h == 0 and j == 0:
                    nc.sync.dma_start(out=aps["dbg_exp"][ci], in_=et[:])
                ets.append(et)
                fill(POP_HEAD // 4 + (1 if ci < POP_HEAD % 4 else 0))
            if pend is not None:
                ph, pets = pend
                emit_ctx_norm(b, j, ph, pets,
                              dbg_tap=(dbg and b == 0 and ph == 0 and j == 0))
            pend = (h, ets)
        ph, pets = pend
        emit_ctx_norm(b, j, ph, pets, dbg_tap=False)

    def mlp_block(b):
        """Per pair: cat transposes -> MLP1 -> Gelu -> MLP2 -> streamed
        out-DMA.  One Gelu/Exp table swap pair per batch."""
        nc.scalar.activation(out=tbl_scr[:], in_=b1_sb[:],
                             func=mybir.ActivationFunctionType.Gelu)
        for pair in range(8):
            cp = catps[b][pair]
            catTs = []
            for j in range(J):
                ptj = ps_big.tile([P, LQ], F32, tag="big", name="ptj")
                catTj = ctp.tile([P, LQ], BF16, tag="ct", name="catTj")
                for lqc in range(4):
                    nc.tensor.matmul(
                        out=ptj[:, lqc * P:(lqc + 1) * P],
                        lhsT=cp[:, lqc, j * P:(j + 1) * P], rhs=ident[:],
                        start=(lqc == 0), stop=(lqc == 3))
                nc.vector.tensor_copy(out=catTj[:], in_=ptj[:])
                catTs.append(catTj)
                fill(POP_MLP)
            ph1p = ps_big.tile([P, LQ], F32, tag="big", name="ph1p")
            for j in range(J):
                nc.tensor.matmul(out=ph1p[:, :], lhsT=w1jd[j][:],
                                 rhs=catTs[j][:],
                                 start=(j == 0), stop=(j == J - 1))
            h1 = h1p.tile([P, LQ], BF16, tag="h1", name="h1")
            nc.scalar.activation(
                out=h1[:], in_=ph1p[:],
                func=mybir.ActivationFunctionType.Gelu, bias=b1_sb[:])
            if dbg and b == 0 and pair == 0:
                nc.sync.dma_start(out=aps["dbg_catT0"][:, :], in_=catTs[0][:])
                nc.sync.dma_start(out=aps["dbg_h1"][:, :], in_=h1[:])
            for lqc in range(4):
                ps2 = ps_att.tile([P, P], F32, tag="att", name="ps2")
                nc.tensor.matmul(
                    out=ps2[:], lhsT=h1[:, lqc * P:(lqc + 1) * P],
                    rhs=w2bd[:], start=True, stop=True)
                stage = ostg.tile([P, P], F32, tag="ostg", name="stage")
                nc.vector.tensor_add(
                    stage[:], ps2[:], b2_bc[:, pair * P:(pair + 1) * P])
                nc.sync.dma_start(
                    out=aps["out"][b, lqc * P:(lqc + 1) * P,
                                   pair * P:(pair + 1) * P],
                    in_=stage[:])
                fill(POP_MLP)
        nc.scalar.activation(out=tbl_scr[:], in_=b1_sb[:],
                             func=mybir.ActivationFunctionType.Exp)
        tmlp[b] = []

    # ================= emission program =================
    dma_qk_consts()
    qin = load_acts(aps["qt_in"][0])
    # table preload for Exp as soon as bq landed (reads bq_sb scratch-wise)
    nc.scalar.activation(out=tbl_scr[:], in_=bq_sb[:, 0:1],
                         func=mybir.ActivationFunctionType.Exp)
    qtp[0] = []
    for t in proj_Q_thunks(qin, qtp[0]):
        t()
    dma_rest_consts()
    kin = load_acts(aps["kt_in"][0, 0])
    kT[(0, 0)] = []
    for t in proj_K_thunks(kin, kT[(0, 0)]):
        t()
    vin = load_acts(aps["vt_in"][0, 0])
    va[(0, 0)] = va_pool.tile([P, 4, H * E], BF16, tag="va", name="va")
    for t in proj_V_thunks(vin, va[(0, 0)]):
        t()

    if dbg:
        for oc in range(8):
            nc.sync.dma_start(out=aps["dbg_qt"][oc], in_=qtp[0][oc][:])
            nc.sync.dma_start(out=aps["dbg_kt0"][oc], in_=kT[(0, 0)][oc][:])
        nc.sync.dma_start(out=aps["dbg_va0"][:, :, :], in_=va[(0, 0)][:])

    def enqueue_kv(b, j):
        kin = load_acts(aps["kt_in"][j, b])
        kT[(b, j)] = []
        fillers.extend(proj_K_thunks(kin, kT[(b, j)]))
        vin = load_acts(aps["vt_in"][j, b])
        va[(b, j)] = va_pool.tile([P, 4, H * E], BF16, tag="va", name="va")
        fillers.extend(proj_V_thunks(vin, va[(b, j)]))

    for b in range(B_LOC):
        catps[b] = [catp.tile([P, 4, J * P], BF16, tag="catp", name="catp")
                    for _ in range(8)]
        tmlp[b] = []
        enqueue_kv(b, 1)
        sweep(b, 0)
        fill(POP_BOUND)
        enqueue_kv(b, 2)
        sweep(b, 1)
        fill(POP_BOUND)
        if b + 1 < B_LOC:
            enqueue_kv(b + 1, 0)
        sweep(b, 2, enq_mlp=True)
        fill(POP_BOUND)
        if b + 1 < B_LOC:
            # q(b+1) only now: its memsets must not enter the Pool queue
            # while qtp(b) slots are still held by this sweep's scores.
            qin = load_acts(aps["qt_in"][b + 1])
            qtp[b + 1] = []
            fillers.extend(proj_Q_thunks(qin, qtp[b + 1]))
        mlp_block(b)
    flush_fillers()

    for p in reversed(ctx_mgr):
        p.__exit__(None, None, None)


_CACHE = {}


def _build(dbg=False):
    key = ("nc", dbg)
    if key in _CACHE:
        return _CACHE[key]
    nc = bacc.Bacc("TRN2", target_bir_lowering=False, debug=False)
    shapes = {
        "qt_in": ([B_LOC, D, LQ], BF16),
        "kt_in": ([J, B_LOC, D, LK], BF16),
        "vt_in": ([J, B_LOC, D, LK], BF16),
        "wqt": ([D, D], BF16),
        "wkt": ([D, D], BF16),
        "wvt": ([D, D], BF16),
        "w1jd": ([J, P, P], BF16),
        "w2bd": ([P, P], BF16),
        "ident": ([P, P], BF16),
        "ones_cols": ([P, 4, H], BF16),
        "bq": ([P, 8], F32),
        "bk": ([P, 8], F32),
        "bv_bc": ([P, D], BF16),
        "b2_bc": ([P, D], F32),
        "b1": ([P, 1], F32),
    }
    aps = {k: nc.dram_tensor(k, s, dt, kind="ExternalInput").ap()
           for k, (s, dt) in shapes.items()}
    aps["out"] = nc.dram_tensor("out", [B_LOC, LQ, D], F32,
                                kind="ExternalOutput").ap()
    if dbg:
        dbg_shapes = {
            "dbg_qt": ([8, P, 512], BF16), "dbg_kt0": ([8, P, 512], BF16),
            "dbg_va0": ([P, 4, H * E], BF16), "dbg_exp": ([4, P, LQ], BF16),
            "dbg_rec": ([P, 4], F32), "dbg_psc": ([P, 4, E], F32),
            "dbg_catT0": ([P, LQ], BF16), "dbg_h1": ([P, LQ], BF16),
        }
        for k, (shp, dt) in dbg_shapes.items():
            aps[k] = nc.dram_tensor(k, shp, dt, kind="ExternalOutput").ap()
    with tile.TileContext(nc) as tc:
        _emit(tc, aps, dbg=dbg)
    nc.compile()
    _CACHE[key] = nc
    return nc


def _prep_in_maps(inputs):
    f32 = np.float32
    bf16 = ml_dtypes.bfloat16
    q = np.ascontiguousarray(np.asarray(inputs["query_states"], f32))
    k = np.ascontiguousarray(np.asarray(inputs["key_states"], f32))
    v = np.ascontiguousarray(np.asarray(inputs["value_states"], f32))
    Wq = np.asarray(inputs["Wq"], f32)
    Wk = np.asarray(inputs["Wk"], f32)
    Wv = np.asarray(inputs["Wv"], f32)
    W1 = np.asarray(inputs["W1"], f32)
    W2 = np.asarray(inputs["W2"], f32)
    bq = np.asarray(inputs["bq"], f32)
    bk = np.asarray(inputs["bk"], f32)
    bv = np.asarray(inputs["bv"], f32)
    b1 = np.asarray(inputs["b1"], f32)
    b2 = np.asarray(inputs["b2"], f32)

    wqt = np.ascontiguousarray(Wq.T).astype(bf16)
    wkt = np.ascontiguousarray(Wk.T).astype(bf16)
    wvt = np.ascontiguousarray(Wv.T).astype(bf16)
    W1T = np.ascontiguousarray(W1.T)                       # [192, 64]
    w1jd = np.zeros((J, P, P), f32)
    for j in range(J):
        blk = W1T[j * HD:(j + 1) * HD]                     # [64, 64]
        w1jd[j, :HD, :HD] = blk
        w1jd[j, HD:, HD:] = blk
    w1jd = w1jd.astype(bf16)
    W2T = np.ascontiguousarray(W2.T)                       # [64, 64]
    w2bd = np.zeros((P, P), f32)
    w2bd[:HD, :HD] = W2T
    w2bd[HD:, HD:] = W2T
    w2bd = w2bd.astype(bf16)
    ident = np.eye(P, dtype=f32).astype(bf16)
    bq_sb = np.ascontiguousarray(bq.reshape(8, P).T).astype(f32)
    bk_sb = np.ascontiguousarray(bk.reshape(8, P).T).astype(f32)
    bv_bc = np.tile(bv, (P, 1)).astype(bf16)
    b2_bc = np.tile(b2, (P, H)).astype(f32)
    b1_col = np.concatenate([b1, b1]).reshape(P, 1).astype(f32)
    ones_cols = np.ones((P, 4, H), f32).astype(bf16)

    qt_all = np.ascontiguousarray(q.transpose(0, 2, 1)).astype(bf16)
    kt_all = np.ascontiguousarray(k.transpose(0, 1, 3, 2)).astype(bf16)
    vt_all = np.ascontiguousarray(v.transpose(0, 1, 3, 2)).astype(bf16)

    in_maps = []
    for c in range(N_CORES):
        sl = slice(c * B_LOC, (c + 1) * B_LOC)
        in_maps.append({
            "qt_in": np.ascontiguousarray(qt_all[sl]),
            "kt_in": np.ascontiguousarray(kt_all[:, sl]),
            "vt_in": np.ascontiguousarray(vt_all[:, sl]),
            "wqt": wqt, "wkt": wkt, "wvt": wvt,
            "w1jd": w1jd, "w2bd": w2bd, "ident": ident,
            "ones_cols": ones_cols,
            "bq": bq_sb, "bk": bk_sb, "bv_bc": bv_bc,
            "b2_bc": b2_bc, "b1": b1_col,
        })
    return in_maps


def kernel(**inputs):
    nc = _build()
    in_maps = _prep_in_maps(inputs)
    res = run_bass_kernel_spmd(nc, in_maps, core_ids=list(range(N_CORES)))
    out = np.concatenate([res.results[i]["out"] for i in range(N_CORES)], axis=0)
    return out.astype(np.float32)


# revision 12
# speedup vs baseline: 1.0626x; 1.0249x over previous
"""Trainium2 Bass kernel for nn_MeshCrossAttention (mesh cross-attention + per-head MLP).

Sharding: data-parallel over batch B=16 -> 2 batches per NeuronCore, 8 cores,
no collectives.

v3 design (vs the 488us v2): the v2 kernel ran as serial per-batch phases:
projections (PE-bound, ScalarE idle ~100us/b) then attention (ScalarE
exp-bound at 100%, PE at ~80%).  Trace analysis: PE stream floor is ~339us
(matmul out-cols at 2.4GHz), ScalarE exp floor ~245us (578ns per [128,512]
exp tile, steady).  So the whole kernel is restructured as ONE software
pipeline where the PE never idles and exp overlaps everything:

  - attention is J-OUTER: sweep j=0..2 over all 16 heads per batch.  kT/va
    live per-j only, which frees enough SBUF to overlap the NEXT batch's
    projections with the current batch's attention.
  - all projection matmul groups after (q,k0,v0) of b0 are emitted as
    FILLER thunks from one global FIFO, popped between score matmuls at a
    tuned rate, so the PE streams projections while ScalarE exps scores.
  - ctx per (head,j) accumulates into ONE psum bank [128,4lqc,65] (ones
    column = softmax denominator per partition, as v2).  normalize is one
    reciprocal (DVE) + one fused broadcast-mult on GPSIMD (Pool engine,
    otherwise idle) into per-pair cat tiles [128,4,384].
  - during the last sweep (j=2) of each batch, the pair transposes + MLP1
    are enqueued as fillers right after each pair's normalize, so the
    filler queue never runs dry; gelu runs as one batch per b (2 act-table
    swaps per b instead of ~8).
  - mlp2 outputs stream to DRAM per [128,128] chunk (no big ost staging,
    no out-DMA tail).
"""
import math
import sys

import numpy as np

if "/opt/trn_rl_repo" not in sys.path:
    sys.path.insert(0, "/opt/trn_rl_repo")

import ml_dtypes  # noqa: E402

import concourse.bass as bass  # noqa: E402
import concourse.tile as tile  # noqa: E402
from concourse import bacc, mybir  # noqa: E402
from concourse.bass_utils import run_bass_kernel_spmd  # noqa: E402

F32 = mybir.dt.float32
BF16 = mybir.dt.bfloat16

D, H, HD, J = 1024, 16, 64, 3
B, LQ, LK = 16, 512, 512
P = 128
N_CORES = 8
B_LOC = B // N_CORES  # 2
E = HD + 1            # 65: head stripe width in va (ones column at HD)

# filler pops per emission point (tuned against the profile)
POP_HEAD = 8          # after each head's 4 score matmuls
POP_BOUND = 24        # at sweep boundaries
POP_MLP = 3           # between MLP-block emissions


def _emit(tc, aps, dbg=False):
    nc = tc.nc
    ctx_mgr = []

    def pool(name, bufs, space="SBUF"):
        p = tc.tile_pool(name=name, bufs=bufs, space=space)
        ctx_mgr.append(p)
        return p.__enter__()

    const = pool("const", 1)
    ain = pool("ain", 12)          # streamed activation chunks [128, 512] bf16
    qt_pool = pool("qt", 32)       # zero-padded per-head qT tiles [128, 512]
    kt_pool = pool("kt", 24)
    va_pool = pool("va", 3)
    expp = pool("expp", 10)
    recp = pool("recp", 4)
    catp = pool("catp", 8)         # pair cat tiles [128, 4, 384]
    ctp = pool("ctp", 8)           # catTj pair tiles [128, 512]
    h1p = pool("h1p", 4)
    ostg = pool("ostg", 6)         # mlp2 out staging [128, 128] f32

    ps_big = pool("ps_big", 2, "PSUM")   # proj accum + cat transposes + mlp1
    ps_att = pool("ps_att", 3, "PSUM")   # scores [128, 512] / mlp2 [128, 128]
    ps_ctx = pool("ps_ctx", 3, "PSUM")   # ctx per (h,j): [128, 4, 65]

    # ---------------- resident constants ----------------
    # wq/bq DMAed up front (first projection); the rest deferred so the PE
    # starts as early as possible.
    wq_sb, wk_sb, wv_sb = [], [], []
    for nm, lst in (("wqt", wq_sb), ("wkt", wk_sb), ("wvt", wv_sb)):
        for i in range(8):
            t = const.tile([P, D], BF16, tag=f"{nm}{i}", name=f"{nm}{i}")
            lst.append(t)
    bq_sb = const.tile([P, 8], F32, tag="bq", name="bq_sb")
    w1jd = [const.tile([P, P], BF16, tag=f"w1jd{j}", name=f"w1jd{j}")
            for j in range(J)]
    w2bd = const.tile([P, P], BF16, tag="w2bd", name="w2bd")
    ident = const.tile([P, P], BF16, tag="ident", name="ident")
    bk_sb = const.tile([P, 8], F32, tag="bk", name="bk_sb")
    bv_bc = const.tile([P, D], BF16, tag="bv", name="bv_bc")
    b2_bc = const.tile([P, D], F32, tag="b2", name="b2_bc")
    b1_sb = const.tile([P, 1], F32, tag="b1", name="b1_sb")
    tbl_scr = const.tile([P, 1], F32, tag="tbl", name="tbl_scr")

    def dma_q_consts():
        for i in range(8):
            nc.sync.dma_start(out=wq_sb[i][:], in_=aps["wqt"][i * P:(i + 1) * P, :])
        nc.sync.dma_start(out=bq_sb[:], in_=aps["bq"][:, :])

    def dma_rest_consts():
        nc.sync.dma_start(out=bv_bc[:], in_=aps["bv_bc"][:, :])
        for j in range(J):
            nc.sync.dma_start(out=w1jd[j][:], in_=aps["w1jd"][j])
        nc.sync.dma_start(out=w2bd[:], in_=aps["w2bd"][:, :])
        nc.sync.dma_start(out=ident[:], in_=aps["ident"][:, :])
        nc.sync.dma_start(out=b2_bc[:], in_=aps["b2_bc"][:, :])
        nc.sync.dma_start(out=b1_sb[:], in_=aps["b1"][:, :])

    def load_acts(ap_slice, w_tiles=None, w_ap=None):
        """Stream 8 activation chunks; optionally interleave the matching
        weight-tile DMAs so weight i lands just before activation i is used."""
        ts = []
        for ic in range(8):
            if w_tiles is not None:
                nc.sync.dma_start(out=w_tiles[ic][:],
                                  in_=w_ap[ic * P:(ic + 1) * P, :])
            t = ain.tile([P, 512], BF16, tag="ain", name="act")
            nc.sync.dma_start(out=t[:], in_=ap_slice[ic * P:(ic + 1) * P, :])
            ts.append(t)
        return ts

    # ---------------- projection emitters ----------------
    def proj_K_thunks(x_tiles, out_list):
        """out_list gets 8 tiles [128, 512] = (Wk @ x^T) + bias; returns
        thunk list (65 units: 64 matmuls + 8 adds merged into last units)."""
        thunks = []
        for oc in range(8):
            pss = ps_big.tile([P, 512], F32, tag="big", name="pssk")
            t = kt_pool.tile([P, 512], BF16, tag="kt", name="kt")
            out_list.append(t)

            def mm(ic, oc=oc, pss=pss):
                nc.tensor.matmul(
                    out=pss[:], lhsT=wk_sb[ic][:, oc * P:(oc + 1) * P],
                    rhs=x_tiles[ic][:], start=(ic == 0), stop=(ic == 7))

            def add(oc=oc, pss=pss, t=t):
                nc.vector.tensor_scalar_add(t[:], pss[:], bk_sb[:, oc:oc + 1])

            for ic in range(8):
                thunks.append(lambda ic=ic, f=mm: f(ic))
            thunks.append(lambda f=add: f())
        return thunks

    def proj_Q_thunks(x_tiles, out_list):
        """Zero-padded per-head qT tiles: head h rows at (h%2)*64, other 64
        rows zero (memset on gpsimd)."""
        thunks = []
        for oc in range(8):
            pss = ps_big.tile([P, 512], F32, tag="big", name="pssq")
            te = qt_pool.tile([P, 512], BF16, tag="qt", name="qtp_e")
            to = qt_pool.tile([P, 512], BF16, tag="qt", name="qtp_o")
            out_list.append(te)
            out_list.append(to)

            def mm(ic, oc=oc, pss=pss):
                nc.tensor.matmul(
                    out=pss[:], lhsT=wq_sb[ic][:, oc * P:(oc + 1) * P],
                    rhs=x_tiles[ic][:], start=(ic == 0), stop=(ic == 7))

            def add(oc=oc, pss=pss, te=te, to=to):
                nc.gpsimd.memset(te[HD:P, :], 0.0)
                nc.gpsimd.memset(to[0:HD, :], 0.0)
                nc.vector.tensor_scalar_add(te[0:HD, :], pss[0:HD, :],
                                            bq_sb[0:HD, oc:oc + 1])
                nc.vector.tensor_scalar_add(to[HD:P, :], pss[HD:P, :],
                                            bq_sb[HD:P, oc:oc + 1])

            for ic in range(8):
                thunks.append(lambda ic=ic, f=mm: f(ic))
            thunks.append(lambda f=add: f())
        return thunks

    def proj_V_thunks(x_tiles, va):
        """va [128, 4, H*E]: natural head-interleaved V + ones column."""
        nc.sync.dma_start(
            out=va.rearrange("p c (h e) -> p c h e", e=E)[:, :, :, HD],
            in_=aps["ones_cols"][:, :, :])
        thunks = []
        for half in range(2):
            for nck in range(4):
                pss = ps_big.tile([P, 512], F32, tag="big", name="pssv")

                def mm(ic, pss=pss, half=half, nck=nck):
                    nc.tensor.matmul(
                        out=pss[:],
                        lhsT=x_tiles[ic][:, nck * P:(nck + 1) * P],
                        rhs=wv_sb[ic][:, half * 512:(half + 1) * 512],
                        start=(ic == 0), stop=(ic == 7))

                def add(pss=pss, half=half, nck=nck):
                    dst = va[:, nck, :].rearrange("p (h e) -> p h e", e=E)[
                        :, half * 8:(half + 1) * 8, 0:HD]
                    nc.vector.tensor_tensor(
                        out=dst,
                        in0=pss[:].rearrange("p (h e) -> p h e", e=HD),
                        in1=bv_bc[:, half * 512:(half + 1) * 512].rearrange(
                            "p (h e) -> p h e", e=HD),
                        op=mybir.AluOpType.add)

                for ic in range(8):
                    thunks.append(lambda ic=ic, f=mm: f(ic))
                thunks.append(lambda f=add: f())
        return thunks

    # ---------------- global filler queue ----------------
    fillers = []

    def fill(n):
        for _ in range(n):
            if not fillers:
                return
            fillers.pop(0)()

    def flush_fillers():
        while fillers:
            fillers.pop(0)()

    # ---------------- attention sweep ----------------
    # per-batch persistent state
    qtp = {}    # b -> list of 16 padded q tiles
    kT = {}     # (b, j) -> list of 8 tiles
    va = {}     # (b, j) -> va tile
    catps = {}  # b -> list of 8 pair cat tiles [128, 4, 384]
    tmlp = {}   # b -> list of per-pair (catTs, ph1p) for gelu/mlp2 block

    def emit_ctx_norm(b, j, h, ets, dbg_tap):
        psc = ps_ctx.tile([P, 4, E], F32, tag="ctx", name="psc")
        for ci in range(4):
            for lqc in range(4):
                nc.tensor.matmul(
                    out=psc[:, lqc, :],
                    lhsT=ets[ci][:, lqc * P:(lqc + 1) * P],
                    rhs=va[(b, j)][:, ci, h * E:(h + 1) * E],
                    start=(ci == 0 and lqc == 0),
                    stop=(ci == 3 and lqc == 3))
        rec = recp.tile([P, 4], F32, tag="rec", name="rec")
        nc.vector.reciprocal(rec[:], psc[:, :, HD])
        nc.vector.tensor_tensor(
            out=catps[b][h // 2][:, :, j * P + (h % 2) * HD:
                                 j * P + (h % 2) * HD + HD],
            in0=psc[:, :, 0:HD],
            in1=rec[:].unsqueeze(2).to_broadcast((P, 4, HD)),
            op=mybir.AluOpType.mult)
        if dbg_tap:
            nc.sync.dma_start(out=aps["dbg_rec"][:, :], in_=rec[:])
            nc.sync.dma_start(out=aps["dbg_psc"][:, :, :], in_=psc[:])

    def sweep(b, j, enq_mlp=False):
        """j-outer attention sweep: 16 heads of scores->exp->ctx->normalize
        for mesh set j, popping fillers to keep the PE streaming."""
        pend = None  # (h, ets) awaiting ctx
        for h in range(H):
            ets = []
            for ci in range(4):
                pss = ps_att.tile([P, LQ], F32, tag="att", name="ps_s")
                nc.tensor.matmul(
                    out=pss[:],
                    lhsT=kT[(b, j)][h // 2][:, ci * P:(ci + 1) * P],
                    rhs=qtp[b][h], start=True, stop=True)
                et = expp.tile([P, LQ], BF16, tag="expp", name="et")
                nc.scalar.activation(
                    out=et[:], in_=pss[:],
                    func=mybir.ActivationFunctionType.Exp,
                    scale=1.0 / math.sqrt(HD))
                if dbg and b == 0 and h == 0 and j == 0:
                    nc.sync.dma_start(out=aps["dbg_exp"][ci], in_=et[:])
                ets.append(et)
                fill(POP_HEAD // 4 + (1 if ci < POP_HEAD % 4 else 0))
            if pend is not None:
                ph, pets = pend
                emit_ctx_norm(b, j, ph, pets,
                              dbg_tap=(dbg and b == 0 and ph == 0 and j == 0))
            pend = (h, ets)
        ph, pets = pend
        emit_ctx_norm(b, j, ph, pets, dbg_tap=False)

    def mlp_block(b):
        """Software-pipelined over pairs with lag: transposes(p) overlap
        MLP1/Gelu(p-1) and MLP2(p-2).  One Gelu/Exp table swap per batch."""
        nc.scalar.activation(out=tbl_scr[:], in_=b1_sb[:],
                             func=mybir.ActivationFunctionType.Gelu)
        catTs_all = [None] * 8
        h1_all = [None] * 8

        def stage_T(pair):
            cp = catps[b][pair]
            catTs = []
            for j in range(J):
                ptj = ps_big.tile([P, LQ], F32, tag="big", name="ptj")
                catTj = ctp.tile([P, LQ], BF16, tag="ct", name="catTj")
                for lqc in range(4):
                    nc.tensor.matmul(
                        out=ptj[:, lqc * P:(lqc + 1) * P],
                        lhsT=cp[:, lqc, j * P:(j + 1) * P], rhs=ident[:],
                        start=(lqc == 0), stop=(lqc == 3))
                nc.vector.tensor_copy(out=catTj[:], in_=ptj[:])
                catTs.append(catTj)
                fill(POP_MLP)
            catTs_all[pair] = catTs

        def stage_M1G(pair):
            catTs = catTs_all[pair]
            ph1p = ps_big.tile([P, LQ], F32, tag="big", name="ph1p")
            for j in range(J):
                nc.tensor.matmul(out=ph1p[:, :], lhsT=w1jd[j][:],
                                 rhs=catTs[j][:],
                                 start=(j == 0), stop=(j == J - 1))
            h1 = h1p.tile([P, LQ], BF16, tag="h1", name="h1")
            nc.scalar.activation(
                out=h1[:], in_=ph1p[:],
                func=mybir.ActivationFunctionType.Gelu, bias=b1_sb[:])
            h1_all[pair] = h1
            if dbg and b == 0 and pair == 0:
                nc.sync.dma_start(out=aps["dbg_catT0"][:, :], in_=catTs[0][:])
                nc.sync.dma_start(out=aps["dbg_h1"][:, :], in_=h1[:])

        def stage_M2(pair):
            h1 = h1_all[pair]
            for lqc in range(4):
                ps2 = ps_att.tile([P, P], F32, tag="att", name="ps2")
                nc.tensor.matmul(
                    out=ps2[:], lhsT=h1[:, lqc * P:(lqc + 1) * P],
                    rhs=w2bd[:], start=True, stop=True)
                stage = ostg.tile([P, P], F32, tag="ostg", name="stage")
                nc.vector.tensor_add(
                    stage[:], ps2[:], b2_bc[:, pair * P:(pair + 1) * P])
                nc.sync.dma_start(
                    out=aps["out"][b, lqc * P:(lqc + 1) * P,
                                   pair * P:(pair + 1) * P],
                    in_=stage[:])
                fill(POP_MLP)

        for pair in range(8):
            if pair >= 1:
                stage_M1G(pair - 1)
            if pair >= 2:
                stage_M2(pair - 2)
            stage_T(pair)
        stage_M1G(7)
        stage_M2(6)
        stage_M2(7)
        nc.scalar.activation(out=tbl_scr[:], in_=b1_sb[:],
                             func=mybir.ActivationFunctionType.Exp)
        tmlp[b] = []

    # ================= emission program =================
    dma_q_consts()
    qin = load_acts(aps["qt_in"][0])
    # table preload for Exp as soon as bq landed (reads bq_sb scratch-wise)
    nc.scalar.activation(out=tbl_scr[:], in_=bq_sb[:, 0:1],
                         func=mybir.ActivationFunctionType.Exp)
    qtp[0] = []
    for t in proj_Q_thunks(qin, qtp[0]):
        t()
    kin = load_acts(aps["kt_in"][0, 0], wk_sb, aps["wkt"])
    nc.sync.dma_start(out=bk_sb[:], in_=aps["bk"][:, :])
    kT[(0, 0)] = []
    for t in proj_K_thunks(kin, kT[(0, 0)]):
        t()
    vin = load_acts(aps["vt_in"][0, 0], wv_sb, aps["wvt"])
    dma_rest_consts()
    va[(0, 0)] = va_pool.tile([P, 4, H * E], BF16, tag="va", name="va")
    for t in proj_V_thunks(vin, va[(0, 0)]):
        t()

    if dbg:
        for oc in range(8):
            nc.sync.dma_start(out=aps["dbg_qt"][oc], in_=qtp[0][oc][:])
            nc.sync.dma_start(out=aps["dbg_kt0"][oc], in_=kT[(0, 0)][oc][:])
        nc.sync.dma_start(out=aps["dbg_va0"][:, :, :], in_=va[(0, 0)][:])

    def enqueue_kv(b, j):
        kin = load_acts(aps["kt_in"][j, b])
        kT[(b, j)] = []
        fillers.extend(proj_K_thunks(kin, kT[(b, j)]))
        vin = load_acts(aps["vt_in"][j, b])
        va[(b, j)] = va_pool.tile([P, 4, H * E], BF16, tag="va", name="va")
        fillers.extend(proj_V_thunks(vin, va[(b, j)]))

    for b in range(B_LOC):
        catps[b] = [catp.tile([P, 4, J * P], BF16, tag="catp", name="catp")
                    for _ in range(8)]
        tmlp[b] = []
        enqueue_kv(b, 1)
        sweep(b, 0)
        fill(POP_BOUND)
        enqueue_kv(b, 2)
        sweep(b, 1)
        fill(POP_BOUND)
        if b + 1 < B_LOC:
            qin = load_acts(aps["qt_in"][b + 1])
            qtp[b + 1] = []
            fillers.extend(proj_Q_thunks(qin, qtp[b + 1]))
            enqueue_kv(b + 1, 0)
        sweep(b, 2, enq_mlp=True)
        fill(POP_BOUND)
        mlp_block(b)
    flush_fillers()

    for p in reversed(ctx_mgr):
        p.__exit__(None, None, None)


_CACHE = {}


def _build(dbg=False):
    key = ("nc", dbg)
    if key in _CACHE:
        return _CACHE[key]
    nc = bacc.Bacc("TRN2", target_bir_lowering=False, debug=False)
    shapes = {
        "qt_in": ([B_LOC, D, LQ], BF16),
        "kt_in": ([J, B_LOC, D, LK], BF16),
        "vt_in": ([J, B_LOC, D, LK], BF16),
        "wqt": ([D, D], BF16),
        "wkt": ([D, D], BF16),
        "wvt": ([D, D], BF16),
        "w1jd": ([J, P, P], BF16),
        "w2bd": ([P, P], BF16),
        "ident": ([P, P], BF16),
        "ones_cols": ([P, 4, H], BF16),
        "bq": ([P, 8], F32),
        "bk": ([P, 8], F32),
        "bv_bc": ([P, D], BF16),
        "b2_bc": ([P, D], F32),
        "b1": ([P, 1], F32),
    }
    aps = {k: nc.dram_tensor(k, s, dt, kind="ExternalInput").ap()
           for k, (s, dt) in shapes.items()}
    aps["out"] = nc.dram_tensor("out", [B_LOC, LQ, D], F32,
                                kind="ExternalOutput").ap()
    if dbg:
        dbg_shapes = {
            "dbg_qt": ([8, P, 512], BF16), "dbg_kt0": ([8, P, 512], BF16),
            "dbg_va0": ([P, 4, H * E], BF16), "dbg_exp": ([4, P, LQ], BF16),
            "dbg_rec": ([P, 4], F32), "dbg_psc": ([P, 4, E], F32),
            "dbg_catT0": ([P, LQ], BF16), "dbg_h1": ([P, LQ], BF16),
        }
        for k, (shp, dt) in dbg_shapes.items():
            aps[k] = nc.dram_tensor(k, shp, dt, kind="ExternalOutput").ap()
    with tile.TileContext(nc) as tc:
        _emit(tc, aps, dbg=dbg)
    nc.compile()
    _CACHE[key] = nc
    return nc


def _prep_in_maps(inputs):
    f32 = np.float32
    bf16 = ml_dtypes.bfloat16
    q = np.ascontiguousarray(np.asarray(inputs["query_states"], f32))
    k = np.ascontiguousarray(np.asarray(inputs["key_states"], f32))
    v = np.ascontiguousarray(np.asarray(inputs["value_states"], f32))
    Wq = np.asarray(inputs["Wq"], f32)
    Wk = np.asarray(inputs["Wk"], f32)
    Wv = np.asarray(inputs["Wv"], f32)
    W1 = np.asarray(inputs["W1"], f32)
    W2 = np.asarray(inputs["W2"], f32)
    bq = np.asarray(inputs["bq"], f32)
    bk = np.asarray(inputs["bk"], f32)
    bv = np.asarray(inputs["bv"], f32)
    b1 = np.asarray(inputs["b1"], f32)
    b2 = np.asarray(inputs["b2"], f32)

    wqt = np.ascontiguousarray(Wq.T).astype(bf16)
    wkt = np.ascontiguousarray(Wk.T).astype(bf16)
    wvt = np.ascontiguousarray(Wv.T).astype(bf16)
    W1T = np.ascontiguousarray(W1.T)                       # [192, 64]
    w1jd = np.zeros((J, P, P), f32)
    for j in range(J):
        blk = W1T[j * HD:(j + 1) * HD]                     # [64, 64]
        w1jd[j, :HD, :HD] = blk
        w1jd[j, HD:, HD:] = blk
    w1jd = w1jd.astype(bf16)
    W2T = np.ascontiguousarray(W2.T)                       # [64, 64]
    w2bd = np.zeros((P, P), f32)
    w2bd[:HD, :HD] = W2T
    w2bd[HD:, HD:] = W2T
    w2bd = w2bd.astype(bf16)
    ident = np.eye(P, dtype=f32).astype(bf16)
    bq_sb = np.ascontiguousarray(bq.reshape(8, P).T).astype(f32)
    bk_sb = np.ascontiguousarray(bk.reshape(8, P).T).astype(f32)
    bv_bc = np.tile(bv, (P, 1)).astype(bf16)
    b2_bc = np.tile(b2, (P, H)).astype(f32)
    b1_col = np.concatenate([b1, b1]).reshape(P, 1).astype(f32)
    ones_cols = np.ones((P, 4, H), f32).astype(bf16)

    qt_all = np.ascontiguousarray(q.transpose(0, 2, 1)).astype(bf16)
    kt_all = np.ascontiguousarray(k.transpose(0, 1, 3, 2)).astype(bf16)
    vt_all = np.ascontiguousarray(v.transpose(0, 1, 3, 2)).astype(bf16)

    in_maps = []
    for c in range(N_CORES):
        sl = slice(c * B_LOC, (c + 1) * B_LOC)
        in_maps.append({
            "qt_in": np.ascontiguousarray(qt_all[sl]),
            "kt_in": np.ascontiguousarray(kt_all[:, sl]),
            "vt_in": np.ascontiguousarray(vt_all[:, sl]),
            "wqt": wqt, "wkt": wkt, "wvt": wvt,
            "w1jd": w1jd, "w2bd": w2bd, "ident": ident,
            "ones_cols": ones_cols,
            "bq": bq_sb, "bk": bk_sb, "bv_bc": bv_bc,
            "b2_bc": b2_bc, "b1": b1_col,
        })
    return in_maps


def kernel(**inputs):
    nc = _build()
    in_maps = _prep_in_maps(inputs)
    res = run_bass_kernel_spmd(nc, in_maps, core_ids=list(range(N_CORES)))
    out = np.concatenate([res.results[i]["out"] for i in range(N_CORES)], axis=0)
    return out.astype(np.float32)


# revision 13
# speedup vs baseline: 1.1708x; 1.1018x over previous
"""Trainium2 Bass kernel for nn_MeshCrossAttention (mesh cross-attention + per-head MLP).

Sharding: data-parallel over batch B=16 -> 2 batches per NeuronCore, 8 cores,
no collectives.

v3 design (vs the 488us v2): the v2 kernel ran as serial per-batch phases:
projections (PE-bound, ScalarE idle ~100us/b) then attention (ScalarE
exp-bound at 100%, PE at ~80%).  Trace analysis: PE stream floor is ~339us
(matmul out-cols at 2.4GHz), ScalarE exp floor ~245us (578ns per [128,512]
exp tile, steady).  So the whole kernel is restructured as ONE software
pipeline where the PE never idles and exp overlaps everything:

  - attention is J-OUTER: sweep j=0..2 over all 16 heads per batch.  kT/va
    live per-j only, which frees enough SBUF to overlap the NEXT batch's
    projections with the current batch's attention.
  - all projection matmul groups after (q,k0,v0) of b0 are emitted as
    FILLER thunks from one global FIFO, popped between score matmuls at a
    tuned rate, so the PE streams projections while ScalarE exps scores.
  - ctx per (head,j) accumulates into ONE psum bank [128,4lqc,65] (ones
    column = softmax denominator per partition, as v2).  normalize is one
    reciprocal (DVE) + one fused broadcast-mult on GPSIMD (Pool engine,
    otherwise idle) into per-pair cat tiles [128,4,384].
  - during the last sweep (j=2) of each batch, the pair transposes + MLP1
    are enqueued as fillers right after each pair's normalize, so the
    filler queue never runs dry; gelu runs as one batch per b (2 act-table
    swaps per b instead of ~8).
  - mlp2 outputs stream to DRAM per [128,128] chunk (no big ost staging,
    no out-DMA tail).
"""
import math
import sys

import numpy as np

if "/opt/trn_rl_repo" not in sys.path:
    sys.path.insert(0, "/opt/trn_rl_repo")

import ml_dtypes  # noqa: E402

import concourse.bass as bass  # noqa: E402
import concourse.tile as tile  # noqa: E402
from concourse import bacc, mybir  # noqa: E402
from concourse.bass_utils import run_bass_kernel_spmd  # noqa: E402

F32 = mybir.dt.float32
BF16 = mybir.dt.bfloat16

D, H, HD, J = 1024, 16, 64, 3
B, LQ, LK = 16, 512, 512
P = 128
N_CORES = 8
B_LOC = B // N_CORES  # 2
E = HD + 1            # 65: head stripe width in va (ones column at HD)

# filler pops per emission point (tuned against the profile)
POP_HEAD = 8          # after each head's 4 score matmuls
POP_BOUND = 24        # at sweep boundaries
POP_MLP = 3           # between MLP-block emissions


def _emit(tc, aps, dbg=False):
    nc = tc.nc
    ctx_mgr = []

    def pool(name, bufs, space="SBUF"):
        p = tc.tile_pool(name=name, bufs=bufs, space=space)
        ctx_mgr.append(p)
        return p.__enter__()

    const = pool("const", 1)
    ain = pool("ain", 16)          # streamed activation chunks [128, 512] bf16
    qt_pool = pool("qt", 32)       # zero-padded per-head qT tiles [128, 512]
    kt_pool = pool("kt", 24)
    va_pool = pool("va", 3)
    expp = pool("expp", 10)
    recp = pool("recp", 4)
    catp = pool("catp", 8)         # pair cat tiles [128, 4, 384]
    ctp = pool("ctp", 8)           # catTj pair tiles [128, 512]
    h1p = pool("h1p", 4)
    ostg = pool("ostg", 6)         # mlp2 out staging [128, 128] f32

    ps_big = pool("ps_big", 2, "PSUM")   # proj accum + cat transposes + mlp1
    ps_att = pool("ps_att", 3, "PSUM")   # scores [128, 512] / mlp2 [128, 128]
    ps_ctx = pool("ps_ctx", 3, "PSUM")   # ctx per (h,j): [128, 4, 65]

    # ---------------- resident constants ----------------
    # wq/bq DMAed up front (first projection); the rest deferred so the PE
    # starts as early as possible.
    wq_sb, wk_sb, wv_sb = [], [], []
    for nm, lst in (("wqt", wq_sb), ("wkt", wk_sb), ("wvt", wv_sb)):
        for i in range(8):
            t = const.tile([P, D], BF16, tag=f"{nm}{i}", name=f"{nm}{i}")
            lst.append(t)
    bq_sb = const.tile([P, 8], F32, tag="bq", name="bq_sb")
    w1jd = [const.tile([P, P], BF16, tag=f"w1jd{j}", name=f"w1jd{j}")
            for j in range(J)]
    w2bd = const.tile([P, P], BF16, tag="w2bd", name="w2bd")
    ident = const.tile([P, P], BF16, tag="ident", name="ident")
    bk_sb = const.tile([P, 8], F32, tag="bk", name="bk_sb")
    bv_bc = const.tile([P, D], BF16, tag="bv", name="bv_bc")
    b2_bc = const.tile([P, D], F32, tag="b2", name="b2_bc")
    b1_sb = const.tile([P, 1], F32, tag="b1", name="b1_sb")
    tbl_scr = const.tile([P, 1], F32, tag="tbl", name="tbl_scr")

    def dma_q_consts():
        for i in range(8):
            nc.sync.dma_start(out=wq_sb[i][:], in_=aps["wqt"][i * P:(i + 1) * P, :])
        nc.sync.dma_start(out=bq_sb[:], in_=aps["bq"][:, :])

    def dma_rest_consts():
        nc.sync.dma_start(out=bv_bc[:], in_=aps["bv_bc"][:, :])
        for j in range(J):
            nc.sync.dma_start(out=w1jd[j][:], in_=aps["w1jd"][j])
        nc.sync.dma_start(out=w2bd[:], in_=aps["w2bd"][:, :])
        nc.sync.dma_start(out=ident[:], in_=aps["ident"][:, :])
        nc.sync.dma_start(out=b2_bc[:], in_=aps["b2_bc"][:, :])
        nc.sync.dma_start(out=b1_sb[:], in_=aps["b1"][:, :])

    def load_acts(ap_slice, w_tiles=None, w_ap=None):
        """Stream 8 activation chunks; optionally interleave the matching
        weight-tile DMAs so weight i lands just before activation i is used."""
        ts = []
        for ic in range(8):
            if w_tiles is not None:
                nc.sync.dma_start(out=w_tiles[ic][:],
                                  in_=w_ap[ic * P:(ic + 1) * P, :])
            t = ain.tile([P, 512], BF16, tag="ain", name="act")
            nc.sync.dma_start(out=t[:], in_=ap_slice[ic * P:(ic + 1) * P, :])
            ts.append(t)
        return ts

    # ---------------- projection emitters ----------------
    def proj_K_thunks(x_tiles, out_list):
        """out_list gets 8 tiles [128, 512] = (Wk @ x^T) + bias; returns
        thunk list (65 units: 64 matmuls + 8 adds merged into last units)."""
        thunks = []
        for oc in range(8):
            pss = ps_big.tile([P, 512], F32, tag="big", name="pssk")
            t = kt_pool.tile([P, 512], BF16, tag="kt", name="kt")
            out_list.append(t)

            def mm(ic, oc=oc, pss=pss):
                nc.tensor.matmul(
                    out=pss[:], lhsT=wk_sb[ic][:, oc * P:(oc + 1) * P],
                    rhs=x_tiles[ic][:], start=(ic == 0), stop=(ic == 7))

            def add(oc=oc, pss=pss, t=t):
                nc.vector.tensor_scalar_add(t[:], pss[:], bk_sb[:, oc:oc + 1])

            for ic in range(8):
                thunks.append(lambda ic=ic, f=mm: f(ic))
            thunks.append(lambda f=add: f())
        return thunks

    def proj_Q_thunks(x_tiles, out_list):
        """Zero-padded per-head qT tiles: head h rows at (h%2)*64, other 64
        rows zero (memset on gpsimd)."""
        thunks = []
        for oc in range(8):
            pss = ps_big.tile([P, 512], F32, tag="big", name="pssq")
            te = qt_pool.tile([P, 512], BF16, tag="qt", name="qtp_e")
            to = qt_pool.tile([P, 512], BF16, tag="qt", name="qtp_o")
            out_list.append(te)
            out_list.append(to)

            def mm(ic, oc=oc, pss=pss):
                nc.tensor.matmul(
                    out=pss[:], lhsT=wq_sb[ic][:, oc * P:(oc + 1) * P],
                    rhs=x_tiles[ic][:], start=(ic == 0), stop=(ic == 7))

            def add(oc=oc, pss=pss, te=te, to=to):
                nc.gpsimd.memset(te[HD:P, :], 0.0)
                nc.gpsimd.memset(to[0:HD, :], 0.0)
                nc.vector.tensor_scalar_add(te[0:HD, :], pss[0:HD, :],
                                            bq_sb[0:HD, oc:oc + 1])
                nc.vector.tensor_scalar_add(to[HD:P, :], pss[HD:P, :],
                                            bq_sb[HD:P, oc:oc + 1])

            for ic in range(8):
                thunks.append(lambda ic=ic, f=mm: f(ic))
            thunks.append(lambda f=add: f())
        return thunks

    def proj_V_thunks(x_tiles, va):
        """va [128, 4, H*E]: natural head-interleaved V + ones column."""
        nc.sync.dma_start(
            out=va.rearrange("p c (h e) -> p c h e", e=E)[:, :, :, HD],
            in_=aps["ones_cols"][:, :, :])
        thunks = []
        for half in range(2):
            for nck in range(4):
                pss = ps_big.tile([P, 512], F32, tag="big", name="pssv")

                def mm(ic, pss=pss, half=half, nck=nck):
                    nc.tensor.matmul(
                        out=pss[:],
                        lhsT=x_tiles[ic][:, nck * P:(nck + 1) * P],
                        rhs=wv_sb[ic][:, half * 512:(half + 1) * 512],
                        start=(ic == 0), stop=(ic == 7))

                def add(pss=pss, half=half, nck=nck):
                    dst = va[:, nck, :].rearrange("p (h e) -> p h e", e=E)[
                        :, half * 8:(half + 1) * 8, 0:HD]
                    nc.vector.tensor_tensor(
                        out=dst,
                        in0=pss[:].rearrange("p (h e) -> p h e", e=HD),
                        in1=bv_bc[:, half * 512:(half + 1) * 512].rearrange(
                            "p (h e) -> p h e", e=HD),
                        op=mybir.AluOpType.add)

                for ic in range(8):
                    thunks.append(lambda ic=ic, f=mm: f(ic))
                thunks.append(lambda f=add: f())
        return thunks

    # ---------------- global filler queue ----------------
    fillers = []

    def fill(n):
        for _ in range(n):
            if not fillers:
                return
            fillers.pop(0)()

    def flush_fillers():
        while fillers:
            fillers.pop(0)()

    # ---------------- attention sweep ----------------
    # per-batch persistent state
    qtp = {}    # b -> list of 16 padded q tiles
    kT = {}     # (b, j) -> list of 8 tiles
    va = {}     # (b, j) -> va tile
    catps = {}  # b -> list of 8 pair cat tiles [128, 4, 384]
    tmlp = {}   # b -> list of per-pair (catTs, ph1p) for gelu/mlp2 block

    def emit_ctx_norm(b, j, h, ets, dbg_tap):
        psc = ps_ctx.tile([P, 4, E], F32, tag="ctx", name="psc")
        for ci in range(4):
            for lqc in range(4):
                nc.tensor.matmul(
                    out=psc[:, lqc, :],
                    lhsT=ets[ci][:, lqc * P:(lqc + 1) * P],
                    rhs=va[(b, j)][:, ci, h * E:(h + 1) * E],
                    start=(ci == 0 and lqc == 0),
                    stop=(ci == 3 and lqc == 3))
        rec = recp.tile([P, 4], F32, tag="rec", name="rec")
        nc.vector.reciprocal(rec[:], psc[:, :, HD])
        nc.vector.tensor_tensor(
            out=catps[b][h // 2][:, :, j * P + (h % 2) * HD:
                                 j * P + (h % 2) * HD + HD],
            in0=psc[:, :, 0:HD],
            in1=rec[:].unsqueeze(2).to_broadcast((P, 4, HD)),
            op=mybir.AluOpType.mult)
        if dbg_tap:
            nc.sync.dma_start(out=aps["dbg_rec"][:, :], in_=rec[:])
            nc.sync.dma_start(out=aps["dbg_psc"][:, :, :], in_=psc[:])

    def sweep(b, j, enq_mlp=False):
        """j-outer attention sweep: 16 heads of scores->exp->ctx->normalize
        for mesh set j, popping fillers to keep the PE streaming."""
        pend = None  # (h, ets) awaiting ctx
        for h in range(H):
            ets = []
            for ci in range(4):
                pss = ps_att.tile([P, LQ], F32, tag="att", name="ps_s")
                nc.tensor.matmul(
                    out=pss[:],
                    lhsT=kT[(b, j)][h // 2][:, ci * P:(ci + 1) * P],
                    rhs=qtp[b][h], start=True, stop=True)
                et = expp.tile([P, LQ], BF16, tag="expp", name="et")
                nc.scalar.activation(
                    out=et[:], in_=pss[:],
                    func=mybir.ActivationFunctionType.Exp,
                    scale=1.0 / math.sqrt(HD))
                if dbg and b == 0 and h == 0 and j == 0:
                    nc.sync.dma_start(out=aps["dbg_exp"][ci], in_=et[:])
                ets.append(et)
                fill(POP_HEAD // 4 + (1 if ci < POP_HEAD % 4 else 0))
            if pend is not None:
                ph, pets = pend
                emit_ctx_norm(b, j, ph, pets,
                              dbg_tap=(dbg and b == 0 and ph == 0 and j == 0))
            pend = (h, ets)
        ph, pets = pend
        emit_ctx_norm(b, j, ph, pets, dbg_tap=False)

    def mlp_block(b):
        """Software-pipelined over pairs with lag: transposes(p) overlap
        MLP1/Gelu(p-1) and MLP2(p-2).  One Gelu/Exp table swap per batch."""
        nc.scalar.activation(out=tbl_scr[:], in_=b1_sb[:],
                             func=mybir.ActivationFunctionType.Gelu)
        catTs_all = [None] * 8
        h1_all = [None] * 8

        def stage_T(pair):
            cp = catps[b][pair]
            catTs = []
            for j in range(J):
                ptj = ps_big.tile([P, LQ], F32, tag="big", name="ptj")
                catTj = ctp.tile([P, LQ], BF16, tag="ct", name="catTj")
                for lqc in range(4):
                    nc.tensor.matmul(
                        out=ptj[:, lqc * P:(lqc + 1) * P],
                        lhsT=cp[:, lqc, j * P:(j + 1) * P], rhs=ident[:],
                        start=(lqc == 0), stop=(lqc == 3))
                if j == 1:
                    nc.scalar.copy(out=catTj[:], in_=ptj[:])
                else:
                    nc.vector.tensor_copy(out=catTj[:], in_=ptj[:])
                catTs.append(catTj)
                fill(POP_MLP)
            catTs_all[pair] = catTs

        def stage_M1G(pair):
            catTs = catTs_all[pair]
            ph1p = ps_big.tile([P, LQ], F32, tag="big", name="ph1p")
            for j in range(J):
                nc.tensor.matmul(out=ph1p[:, :], lhsT=w1jd[j][:],
                                 rhs=catTs[j][:],
                                 start=(j == 0), stop=(j == J - 1))
            h1 = h1p.tile([P, LQ], BF16, tag="h1", name="h1")
            nc.scalar.activation(
                out=h1[:], in_=ph1p[:],
                func=mybir.ActivationFunctionType.Gelu, bias=b1_sb[:])
            h1_all[pair] = h1
            if dbg and b == 0 and pair == 0:
                nc.sync.dma_start(out=aps["dbg_catT0"][:, :], in_=catTs[0][:])
                nc.sync.dma_start(out=aps["dbg_h1"][:, :], in_=h1[:])

        def stage_M2(pair):
            h1 = h1_all[pair]
            for lqc in range(4):
                ps2 = ps_att.tile([P, P], F32, tag="att", name="ps2")
                nc.tensor.matmul(
                    out=ps2[:], lhsT=h1[:, lqc * P:(lqc + 1) * P],
                    rhs=w2bd[:], start=True, stop=True)
                stage = ostg.tile([P, P], F32, tag="ostg", name="stage")
                nc.vector.tensor_add(
                    stage[:], ps2[:], b2_bc[:, pair * P:(pair + 1) * P])
                nc.sync.dma_start(
                    out=aps["out"][b, lqc * P:(lqc + 1) * P,
                                   pair * P:(pair + 1) * P],
                    in_=stage[:])
                fill(POP_MLP)

        for pair in range(8):
            if pair >= 1:
                stage_M1G(pair - 1)
            if pair >= 2:
                stage_M2(pair - 2)
            stage_T(pair)
        stage_M1G(7)
        stage_M2(6)
        stage_M2(7)
        nc.scalar.activation(out=tbl_scr[:], in_=b1_sb[:],
                             func=mybir.ActivationFunctionType.Exp)
        tmlp[b] = []

    # ================= emission program =================
    dma_q_consts()
    qin = load_acts(aps["qt_in"][0])
    # table preload for Exp as soon as bq landed (reads bq_sb scratch-wise)
    nc.scalar.activation(out=tbl_scr[:], in_=bq_sb[:, 0:1],
                         func=mybir.ActivationFunctionType.Exp)
    qtp[0] = []
    for t in proj_Q_thunks(qin, qtp[0]):
        t()
    kin = load_acts(aps["kt_in"][0, 0], wk_sb, aps["wkt"])
    nc.sync.dma_start(out=bk_sb[:], in_=aps["bk"][:, :])
    kT[(0, 0)] = []
    for t in proj_K_thunks(kin, kT[(0, 0)]):
        t()
    vin = load_acts(aps["vt_in"][0, 0], wv_sb, aps["wvt"])
    dma_rest_consts()
    va[(0, 0)] = va_pool.tile([P, 4, H * E], BF16, tag="va", name="va")
    for t in proj_V_thunks(vin, va[(0, 0)]):
        t()

    if dbg:
        for oc in range(8):
            nc.sync.dma_start(out=aps["dbg_qt"][oc], in_=qtp[0][oc][:])
            nc.sync.dma_start(out=aps["dbg_kt0"][oc], in_=kT[(0, 0)][oc][:])
        nc.sync.dma_start(out=aps["dbg_va0"][:, :, :], in_=va[(0, 0)][:])

    def enqueue_kv(b, j):
        kin = load_acts(aps["kt_in"][j, b])
        kT[(b, j)] = []
        fillers.extend(proj_K_thunks(kin, kT[(b, j)]))
        vin = load_acts(aps["vt_in"][j, b])
        va[(b, j)] = va_pool.tile([P, 4, H * E], BF16, tag="va", name="va")
        fillers.extend(proj_V_thunks(vin, va[(b, j)]))

    for b in range(B_LOC):
        catps[b] = [catp.tile([P, 4, J * P], BF16, tag="catp", name="catp")
                    for _ in range(8)]
        tmlp[b] = []
        enqueue_kv(b, 1)
        enqueue_kv(b, 2)
        sweep(b, 0)
        fill(POP_BOUND)
        sweep(b, 1)
        fill(POP_BOUND)
        if b + 1 < B_LOC:
            qin = load_acts(aps["qt_in"][b + 1])
            qtp[b + 1] = []
            fillers.extend(proj_Q_thunks(qin, qtp[b + 1]))
            enqueue_kv(b + 1, 0)
        sweep(b, 2, enq_mlp=True)
        fill(POP_BOUND)
        mlp_block(b)
    flush_fillers()

    for p in reversed(ctx_mgr):
        p.__exit__(None, None, None)


_CACHE = {}


def _build(dbg=False):
    key = ("nc", dbg)
    if key in _CACHE:
        return _CACHE[key]
    nc = bacc.Bacc("TRN2", target_bir_lowering=False, debug=False)
    shapes = {
        "qt_in": ([B_LOC, D, LQ], BF16),
        "kt_in": ([J, B_LOC, D, LK], BF16),
        "vt_in": ([J, B_LOC, D, LK], BF16),
        "wqt": ([D, D], BF16),
        "wkt": ([D, D], BF16),
        "wvt": ([D, D], BF16),
        "w1jd": ([J, P, P], BF16),
        "w2bd": ([P, P], BF16),
        "ident": ([P, P], BF16),
        "ones_cols": ([P, 4, H], BF16),
        "bq": ([P, 8], F32),
        "bk": ([P, 8], F32),
        "bv_bc": ([P, D], BF16),
        "b2_bc": ([P, D], F32),
        "b1": ([P, 1], F32),
    }
    aps = {k: nc.dram_tensor(k, s, dt, kind="ExternalInput").ap()
           for k, (s, dt) in shapes.items()}
    aps["out"] = nc.dram_tensor("out", [B_LOC, LQ, D], F32,
                                kind="ExternalOutput").ap()
    if dbg:
        dbg_shapes = {
            "dbg_qt": ([8, P, 512], BF16), "dbg_kt0": ([8, P, 512], BF16),
            "dbg_va0": ([P, 4, H * E], BF16), "dbg_exp": ([4, P, LQ], BF16),
            "dbg_rec": ([P, 4], F32), "dbg_psc": ([P, 4, E], F32),
            "dbg_catT0": ([P, LQ], BF16), "dbg_h1": ([P, LQ], BF16),
        }
        for k, (shp, dt) in dbg_shapes.items():
            aps[k] = nc.dram_tensor(k, shp, dt, kind="ExternalOutput").ap()
    with tile.TileContext(nc) as tc:
        _emit(tc, aps, dbg=dbg)
    nc.compile()
    _CACHE[key] = nc
    return nc


def _prep_in_maps(inputs):
    f32 = np.float32
    bf16 = ml_dtypes.bfloat16
    q = np.ascontiguousarray(np.asarray(inputs["query_states"], f32))
    k = np.ascontiguousarray(np.asarray(inputs["key_states"], f32))
    v = np.ascontiguousarray(np.asarray(inputs["value_states"], f32))
    Wq = np.asarray(inputs["Wq"], f32)
    Wk = np.asarray(inputs["Wk"], f32)
    Wv = np.asarray(inputs["Wv"], f32)
    W1 = np.asarray(inputs["W1"], f32)
    W2 = np.asarray(inputs["W2"], f32)
    bq = np.asarray(inputs["bq"], f32)
    bk = np.asarray(inputs["bk"], f32)
    bv = np.asarray(inputs["bv"], f32)
    b1 = np.asarray(inputs["b1"], f32)
    b2 = np.asarray(inputs["b2"], f32)

    wqt = np.ascontiguousarray(Wq.T).astype(bf16)
    wkt = np.ascontiguousarray(Wk.T).astype(bf16)
    wvt = np.ascontiguousarray(Wv.T).astype(bf16)
    W1T = np.ascontiguousarray(W1.T)                       # [192, 64]
    w1jd = np.zeros((J, P, P), f32)
    for j in range(J):
        blk = W1T[j * HD:(j + 1) * HD]                     # [64, 64]
        w1jd[j, :HD, :HD] = blk
        w1jd[j, HD:, HD:] = blk
    w1jd = w1jd.astype(bf16)
    W2T = np.ascontiguousarray(W2.T)                       # [64, 64]
    w2bd = np.zeros((P, P), f32)
    w2bd[:HD, :HD] = W2T
    w2bd[HD:, HD:] = W2T
    w2bd = w2bd.astype(bf16)
    ident = np.eye(P, dtype=f32).astype(bf16)
    bq_sb = np.ascontiguousarray(bq.reshape(8, P).T).astype(f32)
    bk_sb = np.ascontiguousarray(bk.reshape(8, P).T).astype(f32)
    bv_bc = np.tile(bv, (P, 1)).astype(bf16)
    b2_bc = np.tile(b2, (P, H)).astype(f32)
    b1_col = np.concatenate([b1, b1]).reshape(P, 1).astype(f32)
    ones_cols = np.ones((P, 4, H), f32).astype(bf16)

    qt_all = np.ascontiguousarray(q.transpose(0, 2, 1)).astype(bf16)
    kt_all = np.ascontiguousarray(k.transpose(0, 1, 3, 2)).astype(bf16)
    vt_all = np.ascontiguousarray(v.transpose(0, 1, 3, 2)).astype(bf16)

    in_maps = []
    for c in range(N_CORES):
        sl = slice(c * B_LOC, (c + 1) * B_LOC)
        in_maps.append({
            "qt_in": np.ascontiguousarray(qt_all[sl]),
            "kt_in": np.ascontiguousarray(kt_all[:, sl]),
            "vt_in": np.ascontiguousarray(vt_all[:, sl]),
            "wqt": wqt, "wkt": wkt, "wvt": wvt,
            "w1jd": w1jd, "w2bd": w2bd, "ident": ident,
            "ones_cols": ones_cols,
            "bq": bq_sb, "bk": bk_sb, "bv_bc": bv_bc,
            "b2_bc": b2_bc, "b1": b1_col,
        })
    return in_maps


def kernel(**inputs):
    nc = _build()
    in_maps = _prep_in_maps(inputs)
    res = run_bass_kernel_spmd(nc, in_maps, core_ids=list(range(N_CORES)))
    out = np.concatenate([res.results[i]["out"] for i in range(N_CORES)], axis=0)
    return out.astype(np.float32)


# revision 55
# speedup vs baseline: 1.2164x; 1.0389x over previous
"""Trainium2 Bass kernel for nn_MeshCrossAttention (mesh cross-attention + per-head MLP).

Sharding: data-parallel over batch B=16 -> 2 batches per NeuronCore, 8 cores,
no collectives.

v3 design (vs the 488us v2): the v2 kernel ran as serial per-batch phases:
projections (PE-bound, ScalarE idle ~100us/b) then attention (ScalarE
exp-bound at 100%, PE at ~80%).  Trace analysis: PE stream floor is ~339us
(matmul out-cols at 2.4GHz), ScalarE exp floor ~245us (578ns per [128,512]
exp tile, steady).  So the whole kernel is restructured as ONE software
pipeline where the PE never idles and exp overlaps everything:

  - attention is J-OUTER: sweep j=0..2 over all 16 heads per batch.  kT/va
    live per-j only, which frees enough SBUF to overlap the NEXT batch's
    projections with the current batch's attention.
  - all projection matmul groups after (q,k0,v0) of b0 are emitted as
    FILLER thunks from one global FIFO, popped between score matmuls at a
    tuned rate, so the PE streams projections while ScalarE exps scores.
  - ctx per (head,j) accumulates into ONE psum bank [128,4lqc,65] (ones
    column = softmax denominator per partition, as v2); ctx trails exp by
    2 heads.  normalize = one DVE reciprocal + one fused broadcast-mult
    into per-pair cat tiles [128,4,384].
  - the back half of the final k/v projections is held in reserve and
    released into the last sweep / the MLP block so those never run dry.
  - MLP blocks are software-pipelined over pairs (transposes(p) overlap
    MLP1/Gelu(p-1), MLP2(p-2)); gelu runs as one batch per b (2 act-table
    swaps per b instead of ~8); mlp2 outputs stream to DRAM per [128,128]
    chunk (no out-DMA tail).

IMPORTANT hardware gotcha found here: the va "ones" columns must be
written by an ENGINE (gpsimd memset), not a scattered 2-byte-strided DMA.
DMA writes into SBUF read-modify-write wider granules, so a scattered DMA
racing the DVE v-proj adds on byte-adjacent columns nondeterministically
clobbers fresh data (CoreSim models DMA atomically and cannot see this).
"""
import math
import sys

import numpy as np

if "/opt/trn_rl_repo" not in sys.path:
    sys.path.insert(0, "/opt/trn_rl_repo")

import ml_dtypes  # noqa: E402

import concourse.bass as bass  # noqa: E402
import concourse.tile as tile  # noqa: E402
from concourse import bacc, mybir  # noqa: E402
from concourse.bass_utils import run_bass_kernel_spmd  # noqa: E402

F32 = mybir.dt.float32
BF16 = mybir.dt.bfloat16

D, H, HD, J = 1024, 16, 64, 3
B, LQ, LK = 16, 512, 512
P = 128
N_CORES = 8
B_LOC = B // N_CORES  # 2
E = HD + 1            # 65: head stripe width in va (ones column at HD)

# filler pops per emission point (tuned against the profile)
POP_HEAD = 8          # after each head's 4 score matmuls
POP_BOUND = 24        # at sweep boundaries
POP_MLP = 3           # between MLP-block emissions


def _emit(tc, aps, dbg=False):
    nc = tc.nc
    ctx_mgr = []

    def pool(name, bufs, space="SBUF"):
        p = tc.tile_pool(name=name, bufs=bufs, space=space)
        ctx_mgr.append(p)
        return p.__enter__()

    const = pool("const", 1)
    ain = pool("ain", 16)          # streamed activation chunks [128, 512] bf16
    qt_pool = pool("qt", 32)       # zero-padded per-head qT tiles [128, 512]
    kt_pool = pool("kt", 24)
    va_pool = pool("va", 3)
    expp = pool("expp", 14)
    recp = pool("recp", 4)
    catp = pool("catp", 8)         # pair cat tiles [128, 4, 384]
    ctp = pool("ctp", 8)           # catTj pair tiles [128, 512]
    h1p = pool("h1p", 4)
    ostg = pool("ostg", 8)         # mlp2 out staging [128, 128] f32

    ps_big = pool("ps_big", 2, "PSUM")   # proj accum + cat transposes + mlp1
    ps_att = pool("ps_att", 4, "PSUM")   # scores [128, 512] / mlp2 [128, 128]
    ps_ctx = pool("ps_ctx", 2, "PSUM")   # ctx per (h,j): [128, 4, 65]

    # ---------------- resident constants ----------------
    # wq/bq DMAed up front (first projection); the rest deferred so the PE
    # starts as early as possible.
    wq_sb, wk_sb, wv_sb = [], [], []
    for nm, lst in (("wqt", wq_sb), ("wkt", wk_sb), ("wvt", wv_sb)):
        for i in range(8):
            t = const.tile([P, D], BF16, tag=f"{nm}{i}", name=f"{nm}{i}")
            lst.append(t)
    bq_sb = const.tile([P, 8], F32, tag="bq", name="bq_sb")
    w1jd = [const.tile([P, P], BF16, tag=f"w1jd{j}", name=f"w1jd{j}")
            for j in range(J)]
    w2bd = const.tile([P, P], BF16, tag="w2bd", name="w2bd")
    ident = const.tile([P, P], BF16, tag="ident", name="ident")
    bk_sb = const.tile([P, 8], F32, tag="bk", name="bk_sb")
    bv_bc = const.tile([P, D], BF16, tag="bv", name="bv_bc")
    b2_bc = const.tile([P, D], F32, tag="b2", name="b2_bc")
    b1_sb = const.tile([P, 1], F32, tag="b1", name="b1_sb")
    tbl_scr = const.tile([P, 1], F32, tag="tbl", name="tbl_scr")

    def dma_rest_consts():
        nc.sync.dma_start(out=bv_bc[:], in_=aps["bv_bc"][:, :])
        for j in range(J):
            nc.sync.dma_start(out=w1jd[j][:], in_=aps["w1jd"][j])
        nc.sync.dma_start(out=w2bd[:], in_=aps["w2bd"][:, :])
        nc.sync.dma_start(out=ident[:], in_=aps["ident"][:, :])
        nc.sync.dma_start(out=b2_bc[:], in_=aps["b2_bc"][:, :])
        nc.sync.dma_start(out=b1_sb[:], in_=aps["b1"][:, :])

    def load_acts(ap_slice, w_tiles=None, w_ap=None):
        """Stream 8 activation chunks; optionally interleave the matching
        weight-tile DMAs so weight i lands just before activation i is used."""
        ts = []
        for ic in range(8):
            if w_tiles is not None:
                nc.sync.dma_start(out=w_tiles[ic][:],
                                  in_=w_ap[ic * P:(ic + 1) * P, :])
            t = ain.tile([P, 512], BF16, tag="ain", name="act")
            nc.sync.dma_start(out=t[:], in_=ap_slice[ic * P:(ic + 1) * P, :])
            ts.append(t)
        return ts

    # ---------------- projection emitters ----------------
    def proj_K_thunks(x_tiles, out_list):
        """out_list gets 8 tiles [128, 512] = (Wk @ x^T) + bias; returns
        thunk list (65 units: 64 matmuls + 8 adds merged into last units)."""
        thunks = []
        for oc in range(8):
            cell = []  # psum alloc deferred to pop time (alloc order == emission order)
            t = kt_pool.tile([P, 512], BF16, tag="kt", name="kt")
            out_list.append(t)

            def mm(ic, oc=oc, cell=cell):
                if ic == 0:
                    cell.append(ps_big.tile([P, 512], F32, tag="big",
                                            name="pssk"))
                nc.tensor.matmul(
                    out=cell[0][:], lhsT=wk_sb[ic][:, oc * P:(oc + 1) * P],
                    rhs=x_tiles[ic][:], start=(ic == 0), stop=(ic == 7))

            def add(oc=oc, cell=cell, t=t):
                nc.vector.tensor_scalar_add(t[:], cell[0][:],
                                            bk_sb[:, oc:oc + 1])

            for ic in range(8):
                thunks.append(lambda ic=ic, f=mm: f(ic))
            thunks.append(lambda f=add: f())
        return thunks

    def proj_Q_thunks(x_tiles, out_list):
        """Zero-padded per-head qT tiles: head h rows at (h%2)*64, other 64
        rows zero (memset on gpsimd)."""
        thunks = []
        for oc in range(8):
            cell = []
            te = qt_pool.tile([P, 512], BF16, tag="qt", name="qtp_e")
            to = qt_pool.tile([P, 512], BF16, tag="qt", name="qtp_o")
            out_list.append(te)
            out_list.append(to)

            def mm(ic, oc=oc, cell=cell):
                if ic == 0:
                    cell.append(ps_big.tile([P, 512], F32, tag="big",
                                            name="pssq"))
                nc.tensor.matmul(
                    out=cell[0][:], lhsT=wq_sb[ic][:, oc * P:(oc + 1) * P],
                    rhs=x_tiles[ic][:], start=(ic == 0), stop=(ic == 7))

            def add(oc=oc, cell=cell, te=te, to=to):
                nc.gpsimd.memset(te[HD:P, :], 0.0)
                nc.gpsimd.memset(to[0:HD, :], 0.0)
                nc.vector.tensor_scalar_add(te[0:HD, :], cell[0][0:HD, :],
                                            bq_sb[0:HD, oc:oc + 1])
                nc.vector.tensor_scalar_add(to[HD:P, :], cell[0][HD:P, :],
                                            bq_sb[HD:P, oc:oc + 1])

            for ic in range(8):
                thunks.append(lambda ic=ic, f=mm: f(ic))
            thunks.append(lambda f=add: f())
        return thunks

    def proj_V_thunks(x_tiles, va):
        """va [128, 4, H*E]: natural head-interleaved V + ones column."""
        nc.gpsimd.memset(
            va.rearrange("p c (h e) -> p c h e", e=E)[:, :, :, HD], 1.0)
        thunks = []
        for half in range(2):
            for nck in range(4):
                cell = []

                def mm(ic, cell=cell, half=half, nck=nck):
                    if ic == 0:
                        cell.append(ps_big.tile([P, 512], F32, tag="big",
                                                name="pssv"))
                    nc.tensor.matmul(
                        out=cell[0][:],
                        lhsT=x_tiles[ic][:, nck * P:(nck + 1) * P],
                        rhs=wv_sb[ic][:, half * 512:(half + 1) * 512],
                        start=(ic == 0), stop=(ic == 7))

                def add(cell=cell, half=half, nck=nck):
                    dst = va[:, nck, :].rearrange("p (h e) -> p h e", e=E)[
                        :, half * 8:(half + 1) * 8, 0:HD]
                    nc.vector.tensor_tensor(
                        out=dst,
                        in0=cell[0][:].rearrange("p (h e) -> p h e", e=HD),
                        in1=bv_bc[:, half * 512:(half + 1) * 512].rearrange(
                            "p (h e) -> p h e", e=HD),
                        op=mybir.AluOpType.add)

                for ic in range(8):
                    thunks.append(lambda ic=ic, f=mm: f(ic))
                thunks.append(lambda f=add: f())
        return thunks

    # ---------------- global filler queue ----------------
    fillers = []

    def fill(n):
        for _ in range(n):
            if not fillers:
                return
            fillers.pop(0)()

    def flush_fillers():
        while fillers:
            fillers.pop(0)()

    # ---------------- attention sweep ----------------
    # per-batch persistent state
    qtp = {}    # b -> list of 16 padded q tiles
    kT = {}     # (b, j) -> list of 8 tiles
    va = {}     # (b, j) -> va tile
    catps = {}  # b -> list of 8 pair cat tiles [128, 4, 384]
    tmlp = {}   # b -> list of per-pair (catTs, ph1p) for gelu/mlp2 block

    def emit_ctx_norm(b, j, h, ets, dbg_tap):
        psc = ps_ctx.tile([P, 4, E], F32, tag="ctx", name="psc")
        for ci in range(4):
            for lqc in range(4):
                nc.tensor.matmul(
                    out=psc[:, lqc, :],
                    lhsT=ets[ci][:, lqc * P:(lqc + 1) * P],
                    rhs=va[(b, j)][:, ci, h * E:(h + 1) * E],
                    start=(ci == 0 and lqc == 0),
                    stop=(ci == 3 and lqc == 3))
        rec = recp.tile([P, 4], F32, tag="rec", name="rec")
        nc.vector.reciprocal(rec[:], psc[:, :, HD])
        nc.vector.tensor_tensor(
            out=catps[b][h // 2][:, :, j * P + (h % 2) * HD:
                                 j * P + (h % 2) * HD + HD],
            in0=psc[:, :, 0:HD],
            in1=rec[:].unsqueeze(2).to_broadcast((P, 4, HD)),
            op=mybir.AluOpType.mult)
        if dbg_tap:
            nc.sync.dma_start(out=aps["dbg_rec"][:, :], in_=rec[:])

    def sweep(b, j, enq_mlp=False, pop_head=POP_HEAD):
        """j-outer attention sweep: 16 heads of scores->exp->ctx->normalize
        for mesh set j, popping fillers to keep the PE streaming."""
        pend = []  # [(h, ets)] awaiting ctx, depth 2
        for h in range(H):
            ets = []
            for ci in range(4):
                pss = ps_att.tile([P, LQ], F32, tag="att", name="ps_s")
                nc.tensor.matmul(
                    out=pss[:],
                    lhsT=kT[(b, j)][h // 2][:, ci * P:(ci + 1) * P],
                    rhs=qtp[b][h], start=True, stop=True)
                et = expp.tile([P, LQ], BF16, tag="expp", name="et")
                nc.scalar.activation(
                    out=et[:], in_=pss[:],
                    func=mybir.ActivationFunctionType.Exp,
                    scale=1.0 / math.sqrt(HD))
                if dbg and b == 0 and h == 0 and j == 0:
                    nc.sync.dma_start(out=aps["dbg_exp"][ci], in_=et[:])
                ets.append(et)
                fill(pop_head // 4 + (1 if ci < pop_head % 4 else 0))
            pend.append((h, ets))
            if len(pend) > 2:
                ph, pets = pend.pop(0)
                emit_ctx_norm(b, j, ph, pets,
                              dbg_tap=(dbg and b == 0 and ph == 0 and j == 0))
        while pend:
            ph, pets = pend.pop(0)
            emit_ctx_norm(b, j, ph, pets, dbg_tap=False)

    def mlp_block(b):
        """Software-pipelined over pairs with lag: transposes(p) overlap
        MLP1/Gelu(p-1) and MLP2(p-2).  One Gelu/Exp table swap per batch."""
        nc.scalar.activation(out=tbl_scr[:], in_=b1_sb[:],
                             func=mybir.ActivationFunctionType.Gelu)
        catTs_all = [None] * 8
        h1_all = [None] * 8

        def stage_T(pair):
            cp = catps[b][pair]
            catTs = []
            for j in range(J):
                ptj = ps_att.tile([P, LQ], F32, tag="att", name="ptj")
                catTj = ctp.tile([P, LQ], BF16, tag="ct", name="catTj")
                for lqc in range(4):
                    nc.tensor.matmul(
                        out=ptj[:, lqc * P:(lqc + 1) * P],
                        lhsT=cp[:, lqc, j * P:(j + 1) * P], rhs=ident[:],
                        start=(lqc == 0), stop=(lqc == 3))
                if j == 1:
                    nc.scalar.copy(out=catTj[:], in_=ptj[:])
                else:
                    nc.vector.tensor_copy(out=catTj[:], in_=ptj[:])
                catTs.append(catTj)
                fill(POP_MLP)
            catTs_all[pair] = catTs

        def stage_M1G(pair):
            catTs = catTs_all[pair]
            ph1p = ps_att.tile([P, LQ], F32, tag="att", name="ph1p")
            for j in range(J):
                nc.tensor.matmul(out=ph1p[:, :], lhsT=w1jd[j][:],
                                 rhs=catTs[j][:],
                                 start=(j == 0), stop=(j == J - 1))
            h1 = h1p.tile([P, LQ], BF16, tag="h1", name="h1")
            nc.scalar.activation(
                out=h1[:], in_=ph1p[:],
                func=mybir.ActivationFunctionType.Gelu, bias=b1_sb[:])
            h1_all[pair] = h1
            if dbg and b == 0 and pair == 0:
                nc.sync.dma_start(out=aps["dbg_catT0"][:, :], in_=catTs[0][:])
                nc.sync.dma_start(out=aps["dbg_h1"][:, :], in_=h1[:])

        def stage_M2(pair):
            h1 = h1_all[pair]
            for lqc in range(4):
                ps2 = ps_att.tile([P, P], F32, tag="att", name="ps2")
                nc.tensor.matmul(
                    out=ps2[:], lhsT=h1[:, lqc * P:(lqc + 1) * P],
                    rhs=w2bd[:], start=True, stop=True)
                stage = ostg.tile([P, P], F32, tag="ostg", name="stage")
                nc.vector.tensor_add(
                    stage[:], ps2[:], b2_bc[:, pair * P:(pair + 1) * P])
                nc.sync.dma_start(
                    out=aps["out"][b, lqc * P:(lqc + 1) * P,
                                   pair * P:(pair + 1) * P],
                    in_=stage[:])
                fill(POP_MLP)

        for pair in range(8):
            if pair >= 1:
                stage_M1G(pair - 1)
            if pair >= 2:
                stage_M2(pair - 2)
            stage_T(pair)
        stage_M2(6)
        stage_M1G(7)
        stage_M2(7)
        nc.scalar.activation(out=tbl_scr[:], in_=b1_sb[:],
                             func=mybir.ActivationFunctionType.Exp)
        tmlp[b] = []

    # ================= emission program =================
    qin = load_acts(aps["qt_in"][0], wq_sb, aps["wqt"])
    nc.sync.dma_start(out=bq_sb[:], in_=aps["bq"][:, :])
    # table preload for Exp as soon as bq landed (reads bq_sb scratch-wise)
    nc.scalar.activation(out=tbl_scr[:], in_=bq_sb[:, 0:1],
                         func=mybir.ActivationFunctionType.Exp)
    qtp[0] = []
    for t in proj_Q_thunks(qin, qtp[0]):
        t()
    kin = load_acts(aps["kt_in"][0, 0], wk_sb, aps["wkt"])
    nc.sync.dma_start(out=bk_sb[:], in_=aps["bk"][:, :])
    kT[(0, 0)] = []
    for t in proj_K_thunks(kin, kT[(0, 0)]):
        t()
    vin = load_acts(aps["vt_in"][0, 0], wv_sb, aps["wvt"])
    dma_rest_consts()
    va[(0, 0)] = va_pool.tile([P, 4, H * E], BF16, tag="va", name="va")
    for t in proj_V_thunks(vin, va[(0, 0)]):
        t()

    if dbg:
        for i in range(16):
            nc.sync.dma_start(out=aps["dbg_qt_a"][i], in_=qtp[0][i][:])
        for oc in range(8):
            nc.sync.dma_start(out=aps["dbg_kt0"][oc], in_=kT[(0, 0)][oc][:])
        nc.sync.dma_start(out=aps["dbg_va0"][:, :, :], in_=va[(0, 0)][:])

    def enqueue_kv(b, j, reserve=None):
        """When reserve is a list, the back half of the thunks (k oc4-7,
        v half1) goes there instead, released later so the filler queue
        does not run dry in the final sweep."""
        kin = load_acts(aps["kt_in"][j, b])
        kT[(b, j)] = []
        kth = proj_K_thunks(kin, kT[(b, j)])
        vin = load_acts(aps["vt_in"][j, b])
        va[(b, j)] = va_pool.tile([P, 4, H * E], BF16, tag="va", name="va")
        vth = proj_V_thunks(vin, va[(b, j)])
        if reserve is None:
            fillers.extend(kth)
            fillers.extend(vth)
        else:
            # front: k oc0-1 + v half0; reserve: k oc2-7 and v half1
            # interleaved per 9-unit group so every group lands before its
            # consumer at 8 pops/head (k-oc_i before scores(h=2i), v-half1
            # complete before the lag-2 ctx(h8) emission at 80 pops).
            fillers.extend(kth[:18])
            fillers.extend(vth[:36])
            kg = [kth[i * 9:(i + 1) * 9] for i in range(2, 8)]
            vg = [vth[i * 9:(i + 1) * 9] for i in range(4, 8)]
            order = [kg[0], vg[0], kg[1], vg[1], kg[2], vg[2],
                     kg[3], vg[3], kg[4], kg[5]]
            for g in order:
                reserve.extend(g)

    for b in range(B_LOC):
        catps[b] = [catp.tile([P, 4, J * P], BF16, tag="catp", name="catp")
                    for _ in range(8)]
        tmlp[b] = []
        reserve = []
        enqueue_kv(b, 1)
        enqueue_kv(b, 2, reserve=(reserve if b + 1 == B_LOC else None))
        sweep(b, 0)
        fill(POP_BOUND)
        sweep(b, 1)
        fill(POP_BOUND)
        mlpres = []
        if b + 1 < B_LOC:
            qin = load_acts(aps["qt_in"][b + 1])
            qtp[b + 1] = []
            fillers.extend(proj_Q_thunks(qin, qtp[b + 1]))
            enqueue_kv(b + 1, 0, reserve=mlpres)
        fillers.extend(reserve)
        sweep(b, 2, enq_mlp=True)
        fill(POP_BOUND)
        if dbg and b == 0:
            flush_fillers()
            for i in range(16):
                nc.sync.dma_start(out=aps["dbg_qt_b"][i], in_=qtp[0][i][:])
            for pr in range(8):
                nc.sync.dma_start(out=aps["dbg_cat_b"][pr],
                                  in_=catps[0][pr][:])
        fillers.extend(mlpres)
        mlp_block(b)
    flush_fillers()

    for p in reversed(ctx_mgr):
        p.__exit__(None, None, None)


_CACHE = {}


def _build(dbg=False):
    key = ("nc", dbg)
    if key in _CACHE:
        return _CACHE[key]
    nc = bacc.Bacc("TRN2", target_bir_lowering=False, debug=False)
    shapes = {
        "qt_in": ([B_LOC, D, LQ], BF16),
        "kt_in": ([J, B_LOC, D, LK], BF16),
        "vt_in": ([J, B_LOC, D, LK], BF16),
        "wqt": ([D, D], BF16),
        "wkt": ([D, D], BF16),
        "wvt": ([D, D], BF16),
        "w1jd": ([J, P, P], BF16),
        "w2bd": ([P, P], BF16),
        "ident": ([P, P], BF16),
        "ones_cols": ([P, 4, H], BF16),
        "bq": ([P, 8], F32),
        "bk": ([P, 8], F32),
        "bv_bc": ([P, D], BF16),
        "b2_bc": ([P, D], F32),
        "b1": ([P, 1], F32),
    }
    aps = {k: nc.dram_tensor(k, s, dt, kind="ExternalInput").ap()
           for k, (s, dt) in shapes.items()}
    aps["out"] = nc.dram_tensor("out", [B_LOC, LQ, D], F32,
                                kind="ExternalOutput").ap()
    if dbg:
        dbg_shapes = {
            "dbg_qt_a": ([16, P, 512], BF16), "dbg_qt_b": ([16, P, 512], BF16),
            "dbg_cat_b": ([8, P, 4, J * P], BF16),
            "dbg_kt0": ([8, P, 512], BF16),
            "dbg_va0": ([P, 4, H * E], BF16), "dbg_exp": ([4, P, LQ], BF16),
            "dbg_rec": ([P, 4], F32), "dbg_psc": ([P, 4, E], F32),
            "dbg_catT0": ([P, LQ], BF16), "dbg_h1": ([P, LQ], BF16),
        }
        for k, (shp, dt) in dbg_shapes.items():
            aps[k] = nc.dram_tensor(k, shp, dt, kind="ExternalOutput").ap()
    with tile.TileContext(nc) as tc:
        _emit(tc, aps, dbg=dbg)
    nc.compile()
    _CACHE[key] = nc
    return nc


def _prep_in_maps(inputs):
    f32 = np.float32
    bf16 = ml_dtypes.bfloat16
    q = np.ascontiguousarray(np.asarray(inputs["query_states"], f32))
    k = np.ascontiguousarray(np.asarray(inputs["key_states"], f32))
    v = np.ascontiguousarray(np.asarray(inputs["value_states"], f32))
    Wq = np.asarray(inputs["Wq"], f32)
    Wk = np.asarray(inputs["Wk"], f32)
    Wv = np.asarray(inputs["Wv"], f32)
    W1 = np.asarray(inputs["W1"], f32)
    W2 = np.asarray(inputs["W2"], f32)
    bq = np.asarray(inputs["bq"], f32)
    bk = np.asarray(inputs["bk"], f32)
    bv = np.asarray(inputs["bv"], f32)
    b1 = np.asarray(inputs["b1"], f32)
    b2 = np.asarray(inputs["b2"], f32)

    wqt = np.ascontiguousarray(Wq.T).astype(bf16)
    wkt = np.ascontiguousarray(Wk.T).astype(bf16)
    wvt = np.ascontiguousarray(Wv.T).astype(bf16)
    W1T = np.ascontiguousarray(W1.T)                       # [192, 64]
    w1jd = np.zeros((J, P, P), f32)
    for j in range(J):
        blk = W1T[j * HD:(j + 1) * HD]                     # [64, 64]
        w1jd[j, :HD, :HD] = blk
        w1jd[j, HD:, HD:] = blk
    w1jd = w1jd.astype(bf16)
    W2T = np.ascontiguousarray(W2.T)                       # [64, 64]
    w2bd = np.zeros((P, P), f32)
    w2bd[:HD, :HD] = W2T
    w2bd[HD:, HD:] = W2T
    w2bd = w2bd.astype(bf16)
    ident = np.eye(P, dtype=f32).astype(bf16)
    bq_sb = np.ascontiguousarray(bq.reshape(8, P).T).astype(f32)
    bk_sb = np.ascontiguousarray(bk.reshape(8, P).T).astype(f32)
    bv_bc = np.tile(bv, (P, 1)).astype(bf16)
    b2_bc = np.tile(b2, (P, H)).astype(f32)
    b1_col = np.concatenate([b1, b1]).reshape(P, 1).astype(f32)
    ones_cols = np.ones((P, 4, H), f32).astype(bf16)

    qt_all = np.ascontiguousarray(q.transpose(0, 2, 1)).astype(bf16)
    kt_all = np.ascontiguousarray(k.transpose(0, 1, 3, 2)).astype(bf16)
    vt_all = np.ascontiguousarray(v.transpose(0, 1, 3, 2)).astype(bf16)

    in_maps = []
    for c in range(N_CORES):
        sl = slice(c * B_LOC, (c + 1) * B_LOC)
        in_maps.append({
            "qt_in": np.ascontiguousarray(qt_all[sl]),
            "kt_in": np.ascontiguousarray(kt_all[:, sl]),
            "vt_in": np.ascontiguousarray(vt_all[:, sl]),
            "wqt": wqt, "wkt": wkt, "wvt": wvt,
            "w1jd": w1jd, "w2bd": w2bd, "ident": ident,
            "ones_cols": ones_cols,
            "bq": bq_sb, "bk": bk_sb, "bv_bc": bv_bc,
            "b2_bc": b2_bc, "b1": b1_col,
        })
    return in_maps


def kernel(**inputs):
    nc = _build()
    in_maps = _prep_in_maps(inputs)
    res = run_bass_kernel_spmd(nc, in_maps, core_ids=list(range(N_CORES)))
    out = np.concatenate([res.results[i]["out"] for i in range(N_CORES)], axis=0)
    return out.astype(np.float32)
